# revision 22
# baseline (speedup 1.0000x reference)
"""CondGraphConv Trainium2 kernel: 8-core SPMD, edge-sharded (i-sorted).

Algebraic restructuring:
    x_e  = Ci[i_e] + Cj[j_e] + relu(sp_e @ Ws + bs) @ Wl_s
    out_e = relu(LN(x_e) * gamma[bid[j_e]] + beta[bid[j_e]])
  where Ci = h @ Wl[:128], Cj = h @ Wl[128:256], h = relu(nf @ Wn + bn).

Phase 1 builds a per-node DRAM table with rows [Cj(128)|gamma(128)|beta(128)]
(768B; gamma/beta expanded per node by an on-chip one-hot matmul over
batch_ids) and keeps each core's own 8 Ci windows resident in SBUF.
Phase 2 processes 128-edge tiles cut at i-window boundaries: the i-side
contribution is a one-hot matmul against the resident Ci window (one-hot
built on-chip: partition_broadcast + is_equal vs an iota column); the j-side
[Cj|gamma|beta] arrives via one indirect DMA per tile (single-column offsets
are the only HW-supported form).  LN mean/var via segmented reduces and an
Act-engine square; normalize+FiLM+relu split across Act/DVE.  Output f16;
host inverse-permutes and upcasts.
"""

import sys
import types

for _p in ("/opt/trn_rl_repo",):
    if _p not in sys.path:
        sys.path.append(_p)

import numpy as np

N, E, B = 6400, 313600, 128
NODE_DIM, COND_DIM, EDGE_DIM = 2048, 1024, 128
S_IN, S_OUT = 8, 30
EPS = 1e-5

NCORES = 8
ECORE = E // NCORES           # 39200 edges per core
NLOC = N // NCORES            # 800 own i-nodes per core
TILE = 128
CH = 16                       # tiles per chunk
GRP = 4                       # tiles per group
F16 = np.float16

_cache = {}


def _axon_shim():
    try:
        import antenv.axon_hooks  # noqa: F401
        return
    except ImportError:
        pass
    try:
        import antenv
        from trn_agent_boot.trn_boot import _ntff_profile_via_ctypes
    except ImportError:
        return
    mod = types.ModuleType("antenv.axon_hooks")
    holder = [None]
    mod.set_axon_ntff_profile_hook = lambda h: holder.__setitem__(0, h)
    mod.get_axon_ntff_profile_hook = lambda: holder[0]
    sys.modules["antenv.axon_hooks"] = mod
    antenv.axon_hooks = mod
    try:
        mod.set_axon_ntff_profile_hook(
            _ntff_profile_via_ctypes("/opt/axon/libaxon_pjrt.so")
        )
    except Exception:
        pass


NWIN = 8


def _plan(inputs):
    """Shard edges by i-range (equal slices of the i-sorted list), cut each
    core's edges into <=128 tiles that never span a 128-node i-window
    (window = global block relative to the core's first block), and build a
    core-uniform tile->window schedule (padded with per-window dummies)."""
    ii = np.asarray(inputs["node_i_ids"]).astype(np.int64)
    order = np.argsort(ii, kind="stable")
    plans = []
    counts = np.zeros((NCORES, NWIN), np.int64)
    for c in range(NCORES):
        eids = order[c * ECORE:(c + 1) * ECORE]
        blk = ii[eids] // 128          # global window id, non-decreasing
        blk0 = int(blk[0])
        tiles = []                      # (start, cnt, win_rel)
        s = 0
        ne = eids.shape[0]
        while s < ne:
            b = blk[s]
            e = min(s + TILE, ne)
            e = s + int(np.searchsorted(blk[s:e], b + 1))
            w = int(b - blk0)
            assert 0 <= w < NWIN
            tiles.append((s, e - s, w))
            counts[c, w] += 1
            s = e
        plans.append({"eids": eids, "blk0": blk0, "tiles": tiles})
    maxcnt = counts.max(axis=0)
    sched = []
    for w in range(NWIN):
        sched.extend([w] * int(maxcnt[w]))
    while len(sched) % CH:
        sched.append(NWIN - 1)
    return plans, sched


def _prep_inputs(inputs, plans, sched):
    nt = len(sched)
    ns = nt * TILE

    nf = np.asarray(inputs["node_feats"], np.float32)
    nfT = np.ascontiguousarray(nf.T.astype(F16))
    wnA = np.asarray(inputs["Wn"], np.float32).astype(F16)
    bnc = np.asarray(inputs["bn"], np.float32).reshape(128, 1)

    KC_GB = COND_DIM // 128 + 1
    KDIM_GB = KC_GB * 128
    cond = np.asarray(inputs["cond_feats"], np.float32)
    condA = np.zeros((KDIM_GB, B), F16)
    condA[:COND_DIM] = cond.T.astype(F16)
    condA[COND_DIM] = 1.0
    wcA = np.zeros((KDIM_GB, 256), F16)
    wcA[:COND_DIM] = np.asarray(inputs["Wc"], np.float32).astype(F16)
    bc_plus = np.asarray(inputs["bc"], np.float32).copy()
    bc_plus[:EDGE_DIM] += 1.0
    wcA[COND_DIM] = bc_plus.astype(F16)

    ws = np.asarray(inputs["Ws"], np.float32).astype(F16)
    bs = np.asarray(inputs["bs"], np.float32).reshape(S_OUT, 1)
    wl = np.asarray(inputs["Wl"], np.float32)
    wlhi = wl[:128].astype(F16)
    wlhj = wl[128:256].astype(F16)
    wls = wl[256:].astype(F16)

    bid = np.asarray(inputs["batch_ids"]).astype(np.int64)
    bidf = bid.astype(F16).reshape(1, N)
    jj_all = np.asarray(inputs["node_j_ids"]).astype(np.int64)
    ii_all = np.asarray(inputs["node_i_ids"]).astype(np.int64)
    spT_full = np.asarray(inputs["spatial_feats"], np.float32).T.astype(F16)

    shared = dict(
        nfT=nfT, wnA=wnA, condA=condA, wcA=wcA, ws=ws, bs=bs, bnc=bnc,
        wlhi=wlhi, wlhj=wlhj, wls=wls, bidf=bidf,
    )
    # schedule slots per window (identical across cores)
    slot_of_win = {}
    for t, w in enumerate(sched):
        slot_of_win.setdefault(w, []).append(t)

    in_maps = []
    for c, p in enumerate(plans):
        eids = p["eids"]
        blk0 = p["blk0"]
        idxJ = np.zeros((128, nt), np.int32)
        ilf = np.full((1, ns), 255.0, F16)   # 255 -> zero one-hot column
        spc = np.zeros((S_IN, ns), F16)
        slotmap = np.full(ns, -1, np.int64)
        used = {w: 0 for w in range(NWIN)}
        for (s, cnt, w) in p["tiles"]:
            t = slot_of_win[w][used[w]]
            used[w] += 1
            sl = eids[s:s + cnt]
            idxJ[:cnt, t] = jj_all[sl]
            ilf[0, t * TILE:t * TILE + cnt] = (ii_all[sl] % 128).astype(F16)
            spc[:, t * TILE:t * TILE + cnt] = spT_full[:, sl]
            slotmap[t * TILE:t * TILE + cnt] = sl
        ciwin = np.zeros((128, NWIN), np.int32)
        for w in range(NWIN):
            g = (blk0 + w) * 128 + np.arange(128)
            ciwin[:, w] = np.minimum(g, N - 1)
        m = dict(shared)
        m["idxJ"] = idxJ
        m["ilf"] = ilf
        m["spc"] = spc
        m["ciwin"] = ciwin
        in_maps.append(m)
        p["slotmap"] = slotmap
    return in_maps


def _build_program(sched):
    import concourse.bass as bass
    import concourse.tile as tile
    from concourse import bacc, mybir
    from contextlib import ExitStack

    f16 = mybir.dt.float16
    f32 = mybir.dt.float32
    i32 = mybir.dt.int32
    i16 = mybir.dt.int16
    AF = mybir.ActivationFunctionType
    OP = mybir.AluOpType

    KC_H = NODE_DIM // 128
    KDIM_H = NODE_DIM
    KC_GB = COND_DIM // 128 + 1
    NB1 = 512
    NCH1 = (N + NB1 - 1) // NB1
    NBLK = N // 128
    nt = len(sched)
    ns = nt * TILE

    nc = bacc.Bacc(
        "TRN2", target_bir_lowering=False, debug=False,
        num_devices=NCORES, num_swdge_queues=1,
    )

    nfT = nc.dram_tensor("nfT", [KDIM_H, N], f16, kind="ExternalInput")
    wnA = nc.dram_tensor("wnA", [KDIM_H, 128], f16, kind="ExternalInput")
    condA = nc.dram_tensor("condA", [KC_GB * 128, B], f16, kind="ExternalInput")
    wcA = nc.dram_tensor("wcA", [KC_GB * 128, 256], f16, kind="ExternalInput")
    ws = nc.dram_tensor("ws", [S_IN, S_OUT], f16, kind="ExternalInput")
    bs = nc.dram_tensor("bs", [S_OUT, 1], f32, kind="ExternalInput")
    bnc = nc.dram_tensor("bnc", [128, 1], f32, kind="ExternalInput")
    wlhi = nc.dram_tensor("wlhi", [128, 128], f16, kind="ExternalInput")
    wlhj = nc.dram_tensor("wlhj", [128, 128], f16, kind="ExternalInput")
    wls = nc.dram_tensor("wls", [S_OUT, 128], f16, kind="ExternalInput")
    bidf = nc.dram_tensor("bidf", [1, N], f16, kind="ExternalInput")
    idxJ = nc.dram_tensor("idxJ", [128, nt], i32, kind="ExternalInput")
    ilf = nc.dram_tensor("ilf", [1, ns], f16, kind="ExternalInput")
    ciwin = nc.dram_tensor("ciwin", [128, NWIN], i32, kind="ExternalInput")
    spc = nc.dram_tensor("spc", [S_IN, ns], f16, kind="ExternalInput")
    out = nc.dram_tensor("out", [ns, 128], f16, kind="ExternalOutput")

    tbl = nc.dram_tensor("tbl", [N, 384], f16)

    with tile.TileContext(nc) as tc:
        with ExitStack() as ctx:
            const = ctx.enter_context(tc.tile_pool(name="const", bufs=1))

            wn_sb = const.tile([128, KC_H * 128], f16)
            for k in range(KC_H):
                nc.sync.dma_start(
                    out=wn_sb[:, k * 128:(k + 1) * 128],
                    in_=wnA.ap()[k * 128:(k + 1) * 128, :],
                )
            cond_sb = const.tile([128, KC_GB * 128], f16)
            wc_sb = const.tile([128, KC_GB * 256], f16)
            for k in range(KC_GB):
                nc.sync.dma_start(
                    out=cond_sb[:, k * 128:(k + 1) * 128],
                    in_=condA.ap()[k * 128:(k + 1) * 128, :],
                )
                nc.sync.dma_start(
                    out=wc_sb[:, k * 256:(k + 1) * 256],
                    in_=wcA.ap()[k * 128:(k + 1) * 128, :],
                )
            ws_sb = const.tile([S_IN, S_OUT], f16)
            nc.sync.dma_start(out=ws_sb[:], in_=ws.ap())
            bs_sb = const.tile([S_OUT, 1], f32)
            nc.sync.dma_start(out=bs_sb[:], in_=bs.ap())
            bn_sb = const.tile([128, 1], f32)
            nc.sync.dma_start(out=bn_sb[:], in_=bnc.ap())
            wlhi_sb = const.tile([128, 128], f16)
            nc.sync.dma_start(out=wlhi_sb[:], in_=wlhi.ap())
            wlhj_sb = const.tile([128, 128], f16)
            nc.sync.dma_start(out=wlhj_sb[:], in_=wlhj.ap())
            wls_sb = const.tile([S_OUT, 128], f16)
            nc.sync.dma_start(out=wls_sb[:], in_=wls.ap())
            bid_sb = const.tile([1, N], f16)
            nc.sync.dma_start(out=bid_sb[:], in_=bidf.ap())
            idxj_sb = const.tile([128, nt], i32)
            nc.sync.dma_start(out=idxj_sb[:], in_=idxJ.ap())
            ciwin_sb = const.tile([128, NWIN], i32)
            nc.sync.dma_start(out=ciwin_sb[:], in_=ciwin.ap())
            eps_sb = const.tile([128, 1], f32)
            nc.vector.memset(eps_sb[:], EPS)
            io16 = const.tile([128, 1], i16)
            nc.gpsimd.iota(io16[:], pattern=[[0, 1]], channel_multiplier=1)
            iof = const.tile([128, 1], f32)
            nc.vector.tensor_copy(iof[:], io16[:])
            gb_sb = const.tile([128, 256], f16)
            ci_loc = const.tile([128, NWIN, 128], f16)

            # ================= phase 1: node table =================
            with ExitStack() as p1:
                w1 = p1.enter_context(tc.tile_pool(name="w1", bufs=2))
                ps1 = p1.enter_context(
                    tc.tile_pool(name="ps1", bufs=1, space="PSUM")
                )

                gb_ps = ps1.tile([128, 256], f32, tag="gbps")
                for k in range(KC_GB):
                    nc.tensor.matmul(
                        out=gb_ps[:],
                        lhsT=cond_sb[:, k * 128:(k + 1) * 128],
                        rhs=wc_sb[:, k * 256:(k + 1) * 256],
                        start=(k == 0), stop=(k == KC_GB - 1),
                    )
                nc.scalar.copy(gb_sb[:], gb_ps[:])

                # per-node batch one-hot source: broadcast bid row
                bidr = const.tile([128, N], f16)
                nc.gpsimd.partition_broadcast(bidr[:], bid_sb[:])
                ohb_all = const.tile([128, NBLK, 128], f16)
                nc.vector.tensor_scalar(
                    out=ohb_all[:].rearrange("p a b -> p (a b)"),
                    in0=bidr[:], scalar1=iof[:], scalar2=None,
                    op0=OP.is_equal,
                )

                ci_dram = nc.dram_tensor("ci_tbl", [N, 128], f16)

                for nb in range(NCH1):
                    n0 = nb * NB1
                    nsz = min(NB1, N - n0)
                    nts = nsz // 128
                    ht_psA = ps1.tile([128, NB1], f32, tag="htpsA", bufs=2)
                    ht_psB = ps1.tile([128, NB1], f32, tag="htpsB", bufs=2)
                    for k4 in range(KC_H // 4):
                        nf_t = w1.tile([128, 4, NB1], f16, tag="nft", bufs=4)
                        nc.sync.dma_start(
                            out=nf_t[:, :, :nsz],
                            in_=nfT.ap()[
                                k4 * 512:(k4 + 1) * 512, n0:n0 + nsz
                            ].rearrange("(a p) n -> p a n", p=128),
                        )
                        for kk in range(4):
                            k = k4 * 4 + kk
                            ps = ht_psA if k % 2 == 0 else ht_psB
                            nc.tensor.matmul(
                                out=ps[:, :nsz],
                                lhsT=wn_sb[:, k * 128:(k + 1) * 128],
                                rhs=nf_t[:, kk, :nsz],
                                start=(k < 2), stop=(k >= KC_H - 2),
                            )
                    ht_b = w1.tile([128, NB1], f32, tag="htb", bufs=2)
                    nc.vector.tensor_copy(ht_b[:, :nsz], ht_psB[:, :nsz])
                    ht_f = w1.tile([128, NB1], f32, tag="htf", bufs=2)
                    nc.vector.tensor_tensor(
                        out=ht_f[:, :nsz], in0=ht_psA[:, :nsz],
                        in1=ht_b[:, :nsz], op=OP.add,
                    )
                    ht_sb = w1.tile([128, NB1], f16, tag="htsb", bufs=3)
                    nc.scalar.activation(
                        ht_sb[:, :nsz], ht_f[:, :nsz], AF.Relu, bias=bn_sb[:]
                    )
                    for st in range(nts):
                        blk = (n0 + st * 128) // 128
                        lhs = ht_sb[:, st * 128:(st + 1) * 128]
                        tb_sb = w1.tile([128, 384], f16, tag="tbsb", bufs=3)
                        nb_ps = ps1.tile([128, 4, 128], f32, tag="nbps", bufs=2)
                        nc.tensor.matmul(
                            out=nb_ps[:, 0, :], lhsT=lhs, rhs=wlhj_sb[:],
                            start=True, stop=True,
                        )
                        nc.tensor.matmul(
                            out=nb_ps[:, 1:3, :].rearrange("p a b -> p (a b)"),
                            lhsT=ohb_all[:, blk, :],
                            rhs=gb_sb[:], start=True, stop=True,
                        )
                        nc.scalar.copy(
                            tb_sb[:],
                            nb_ps[:, 0:3, :].rearrange("p a b -> p (a b)"),
                        )
                        nc.sync.dma_start(
                            out=tbl.ap()[blk * 128:(blk + 1) * 128, :],
                            in_=tb_sb[:],
                        )
                        nc.tensor.matmul(
                            out=nb_ps[:, 3, :], lhsT=lhs, rhs=wlhi_sb[:],
                            start=True, stop=True,
                        )
                        ci_sb = w1.tile([128, 128], f16, tag="cisb", bufs=3)
                        nc.scalar.copy(ci_sb[:], nb_ps[:, 3, :])
                        nc.sync.dma_start(
                            out=ci_dram.ap()[blk * 128:(blk + 1) * 128, :],
                            in_=ci_sb[:],
                        )

                tc.strict_bb_all_engine_barrier()

                # own Ci windows -> SBUF via 8 single-column indirect DMAs
                for wdx in range(NWIN):
                    nc.gpsimd.indirect_dma_start(
                        out=ci_loc[:, wdx, :], out_offset=None,
                        in_=ci_dram.ap(),
                        in_offset=bass.IndirectOffsetOnAxis(
                            ap=ciwin_sb[:, wdx:wdx + 1], axis=0
                        ),
                    )

                tc.strict_bb_all_engine_barrier()

            # ================= phase 2: edges =================
            with ExitStack() as p2:
                w2 = p2.enter_context(tc.tile_pool(name="w2", bufs=2))
                sm = p2.enter_context(tc.tile_pool(name="sm", bufs=4))
                ps_x = p2.enter_context(
                    tc.tile_pool(name="psx", bufs=2, space="PSUM")
                )
                ps_s = p2.enter_context(
                    tc.tile_pool(name="pss", bufs=2, space="PSUM")
                )

                NGRP = nt // GRP
                for g in range(NGRP):
                    t0 = g * GRP
                    if t0 % CH == 0:
                        ils = w2.tile([1, CH * TILE], f16, tag="ils")
                        nc.sync.dma_start(
                            out=ils[:],
                            in_=ilf.ap()[:, t0 * TILE:(t0 + CH) * TILE],
                        )
                        ilr = w2.tile([128, CH * TILE], f16, tag="ilr")
                        nc.gpsimd.partition_broadcast(ilr[:], ils[:])
                        ohi = w2.tile([128, CH, TILE], f16, tag="ohi")
                        nc.vector.tensor_scalar(
                            out=ohi[:].rearrange("p a b -> p (a b)"),
                            in0=ilr[:], scalar1=iof[:], scalar2=None,
                            op0=OP.is_equal,
                        )
                        spt = w2.tile([S_IN, CH * TILE], f16, tag="spt")
                        nc.sync.dma_start(
                            out=spt[:],
                            in_=spc.ap()[:, t0 * TILE:(t0 + CH) * TILE],
                        )
                        cje = w2.tile([128, CH, 384], f16, tag="cje")
                        ob = w2.tile([128, CH, 128], f16, tag="ob")
                    co = t0 % CH

                    for j in range(GRP):
                        nc.gpsimd.indirect_dma_start(
                            out=cje[:, co + j, :], out_offset=None,
                            in_=tbl.ap(),
                            in_offset=bass.IndirectOffsetOnAxis(
                                ap=idxj_sb[:, t0 + j:t0 + j + 1], axis=0
                            ),
                        )

                    s_ps = ps_s.tile([S_OUT, GRP * TILE], f32, tag="sps")
                    nc.tensor.matmul(
                        out=s_ps[:], lhsT=ws_sb[:],
                        rhs=spt[:, co * TILE:(co + GRP) * TILE],
                        start=True, stop=True,
                    )
                    sT = sm.tile([S_OUT, GRP * TILE], f16, tag="sT")
                    nc.scalar.activation(sT[:], s_ps[:], AF.Relu, bias=bs_sb[:])

                    xs_ps = ps_x.tile([128, GRP, 128], f32, tag="xs")
                    for j in range(GRP):
                        nc.tensor.matmul(
                            out=xs_ps[:, j, :],
                            lhsT=ohi[:, co + j, :],
                            rhs=ci_loc[:, sched[t0 + j], :],
                            start=True, stop=False,
                        )
                        nc.tensor.matmul(
                            out=xs_ps[:, j, :],
                            lhsT=sT[:, j * TILE:(j + 1) * TILE],
                            rhs=wls_sb[:], start=False, stop=True,
                        )

                    xsb = sm.tile([128, GRP, 128], f16, tag="xsb")
                    nc.vector.tensor_tensor(
                        out=xsb[:], in0=cje[:, co:co + GRP, 0:128],
                        in1=xs_ps[:], op=OP.add,
                    )
                    sums = sm.tile([128, GRP], f32, tag="sums")
                    nc.vector.tensor_reduce(
                        out=sums[:], in_=xsb[:], axis=mybir.AxisListType.X,
                        op=OP.add,
                    )
                    sq = sm.tile([128, GRP, 128], f16, tag="sq")
                    nc.scalar.square(sq[:], xsb[:])
                    ssq = sm.tile([128, GRP], f32, tag="ssq")
                    nc.vector.tensor_reduce(
                        out=ssq[:], in_=sq[:], axis=mybir.AxisListType.X,
                        op=OP.add,
                    )
                    negmu = sm.tile([128, GRP], f32, tag="negmu")
                    nc.vector.tensor_scalar(
                        out=negmu[:], in0=sums[:], scalar1=-1.0 / 128,
                        scalar2=None, op0=OP.mult,
                    )
                    musq = sm.tile([128, GRP], f32, tag="musq")
                    nc.vector.tensor_tensor(
                        out=musq[:], in0=negmu[:], in1=negmu[:], op=OP.mult,
                    )
                    var = sm.tile([128, GRP], f32, tag="var")
                    nc.vector.tensor_scalar(
                        out=var[:], in0=ssq[:], scalar1=1.0 / 128,
                        scalar2=None, op0=OP.mult,
                    )
                    nc.vector.tensor_tensor(
                        out=var[:], in0=var[:], in1=musq[:], op=OP.subtract,
                    )
                    std = sm.tile([128, GRP], f32, tag="std")
                    nc.scalar.activation(
                        std[:], var[:], AF.Sqrt, bias=eps_sb[:]
                    )
                    rstd = sm.tile([128, GRP], f32, tag="rstd")
                    nc.vector.reciprocal(rstd[:], std[:])
                    nmr = sm.tile([128, GRP], f32, tag="nmr")
                    nc.vector.tensor_tensor(
                        out=nmr[:], in0=negmu[:], in1=rstd[:], op=OP.mult,
                    )
                    xn = sm.tile([128, GRP, 128], f16, tag="xn")
                    for j in range(GRP):
                        if j % 2 == 0:
                            nc.scalar.activation(
                                xn[:, j, :], xsb[:, j, :], AF.Identity,
                                bias=nmr[:, j:j + 1], scale=rstd[:, j:j + 1],
                            )
                        else:
                            nc.vector.tensor_scalar(
                                out=xn[:, j, :], in0=xsb[:, j, :],
                                scalar1=negmu[:, j:j + 1],
                                scalar2=rstd[:, j:j + 1],
                                op0=OP.add, op1=OP.mult,
                            )
                    xf = sm.tile([128, GRP, 128], f16, tag="xf")
                    nc.vector.tensor_tensor(
                        out=xf[:], in0=xn[:], in1=cje[:, co:co + GRP, 128:256],
                        op=OP.mult,
                    )
                    xb = sm.tile([128, GRP, 128], f16, tag="xb")
                    nc.vector.tensor_tensor(
                        out=xb[:], in0=xf[:], in1=cje[:, co:co + GRP, 256:384],
                        op=OP.add,
                    )
                    nc.scalar.activation(ob[:, co:co + GRP, :], xb[:], AF.Relu)
                    if (t0 + GRP) % CH == 0:
                        c0 = (t0 + GRP - CH) * TILE
                        nc.sync.dma_start(
                            out=out.ap()[c0:c0 + CH * TILE, :].rearrange(
                                "(t p) d -> p t d", p=128
                            ),
                            in_=ob[:],
                        )

    nc.compile()
    return nc


def _run(inputs, trace=False):
    _axon_shim()
    from concourse.bass_utils import run_bass_kernel_spmd

    ii = np.asarray(inputs["node_i_ids"])
    key = hash(ii.tobytes())
    if _cache.get("key") != key:
        plans, sched = _plan(inputs)
        _cache.update(
            key=key, plans=plans, sched=sched, nc=_build_program(sched)
        )
    plans, sched, nc = _cache["plans"], _cache["sched"], _cache["nc"]
    in_maps = _prep_inputs(inputs, plans, sched)

    res = run_bass_kernel_spmd(
        nc, in_maps, core_ids=list(range(NCORES)), trace=trace
    )
    full = np.zeros((E, 128), np.float32)
    for c, p in enumerate(plans):
        sm_ = p["slotmap"]
        valid = sm_ >= 0
        full[sm_[valid]] = res.results[c]["out"][valid].astype(np.float32)
    return full, res


def kernel(**inputs):
    full, _ = _run(inputs, trace=False)
    return full.astype(np.float32)


# revision 33
# speedup vs baseline: 1.0805x; 1.0805x over previous
"""CondGraphConv Trainium2 kernel: 8-core SPMD, edge-sharded (i-sorted).

Algebraic restructuring:
    x_e  = Ci[i_e] + Cj[j_e] + relu(sp_e @ Ws + bs) @ Wl_s
    out_e = relu(LN(x_e) * gamma[bid[j_e]] + beta[bid[j_e]])
  where Ci = h @ Wl[:128], Cj = h @ Wl[128:256], h = relu(nf @ Wn + bn).

Phase 1 builds a per-node DRAM table with rows [Cj(128)|gamma(128)|beta(128)]
(768B; gamma/beta expanded per node by an on-chip one-hot matmul over
batch_ids) and keeps each core's own 8 Ci windows resident in SBUF.
Phase 2 processes 128-edge tiles cut at i-window boundaries: the i-side
contribution is a one-hot matmul against the resident Ci window (one-hot
built on-chip: partition_broadcast + is_equal vs an iota column); the j-side
[Cj|gamma|beta] arrives via one indirect DMA per tile (single-column offsets
are the only HW-supported form).  LN mean/var via segmented reduces and an
Act-engine square; normalize+FiLM+relu split across Act/DVE.  Output f16;
host inverse-permutes and upcasts.
"""

import sys
import types

for _p in ("/opt/trn_rl_repo",):
    if _p not in sys.path:
        sys.path.append(_p)

import numpy as np

N, E, B = 6400, 313600, 128
NODE_DIM, COND_DIM, EDGE_DIM = 2048, 1024, 128
S_IN, S_OUT = 8, 30
EPS = 1e-5

NCORES = 8
ECORE = E // NCORES           # 39200 edges per core
NLOC = N // NCORES            # 800 own i-nodes per core
TILE = 128
CH = 16                       # tiles per chunk
GRP = 4                       # tiles per group
F16 = np.float16

_cache = {}


def _axon_shim():
    try:
        import antenv.axon_hooks  # noqa: F401
        return
    except ImportError:
        pass
    try:
        import antenv
        from trn_agent_boot.trn_boot import _ntff_profile_via_ctypes
    except ImportError:
        return
    mod = types.ModuleType("antenv.axon_hooks")
    holder = [None]
    mod.set_axon_ntff_profile_hook = lambda h: holder.__setitem__(0, h)
    mod.get_axon_ntff_profile_hook = lambda: holder[0]
    sys.modules["antenv.axon_hooks"] = mod
    antenv.axon_hooks = mod
    try:
        mod.set_axon_ntff_profile_hook(
            _ntff_profile_via_ctypes("/opt/axon/libaxon_pjrt.so")
        )
    except Exception:
        pass


NWIN = 8


def _plan(inputs):
    """Shard edges by i-range (equal slices of the i-sorted list), cut each
    core's edges into <=128 tiles that never span a 128-node i-window
    (window = global block relative to the core's first block), and build a
    core-uniform tile->window schedule (padded with per-window dummies)."""
    ii = np.asarray(inputs["node_i_ids"]).astype(np.int64)
    order = np.argsort(ii, kind="stable")
    plans = []
    counts = np.zeros((NCORES, NWIN), np.int64)
    for c in range(NCORES):
        eids = order[c * ECORE:(c + 1) * ECORE]
        blk = ii[eids] // 128          # global window id, non-decreasing
        blk0 = int(blk[0])
        tiles = []                      # (start, cnt, win_rel)
        s = 0
        ne = eids.shape[0]
        while s < ne:
            b = blk[s]
            e = min(s + TILE, ne)
            e = s + int(np.searchsorted(blk[s:e], b + 1))
            w = int(b - blk0)
            assert 0 <= w < NWIN
            tiles.append((s, e - s, w))
            counts[c, w] += 1
            s = e
        plans.append({"eids": eids, "blk0": blk0, "tiles": tiles})
    maxcnt = counts.max(axis=0)
    sched = []
    for w in range(NWIN):
        sched.extend([w] * int(maxcnt[w]))
    while len(sched) % CH:
        sched.append(NWIN - 1)
    return plans, sched


def _prep_inputs(inputs, plans, sched):
    nt = len(sched)
    ns = nt * TILE

    nf = np.asarray(inputs["node_feats"], np.float32)
    nfT = np.ascontiguousarray(nf.T.astype(F16))
    wnA = np.asarray(inputs["Wn"], np.float32).astype(F16)
    bnc = np.asarray(inputs["bn"], np.float32).reshape(128, 1)

    KC_GB = COND_DIM // 128 + 1
    KDIM_GB = KC_GB * 128
    cond = np.asarray(inputs["cond_feats"], np.float32)
    condA = np.zeros((KDIM_GB, B), F16)
    condA[:COND_DIM] = cond.T.astype(F16)
    condA[COND_DIM] = 1.0
    wcA = np.zeros((KDIM_GB, 256), F16)
    wcA[:COND_DIM] = np.asarray(inputs["Wc"], np.float32).astype(F16)
    bc_plus = np.asarray(inputs["bc"], np.float32).copy()
    bc_plus[:EDGE_DIM] += 1.0
    wcA[COND_DIM] = bc_plus.astype(F16)

    ws = np.asarray(inputs["Ws"], np.float32).astype(F16)
    bs = np.asarray(inputs["bs"], np.float32).reshape(S_OUT, 1)
    wl = np.asarray(inputs["Wl"], np.float32)
    wlhi = wl[:128].astype(F16)
    wlhj = wl[128:256].astype(F16)
    wls = wl[256:].astype(F16)

    bid = np.asarray(inputs["batch_ids"]).astype(np.int64)
    bidf = bid.astype(F16).reshape(1, N)
    jj_all = np.asarray(inputs["node_j_ids"]).astype(np.int64)
    ii_all = np.asarray(inputs["node_i_ids"]).astype(np.int64)
    spT_full = np.asarray(inputs["spatial_feats"], np.float32).T.astype(F16)

    shared = dict(
        nfT=nfT, wnA=wnA, condA=condA, wcA=wcA, ws=ws, bs=bs, bnc=bnc,
        wlhi=wlhi, wlhj=wlhj, wls=wls, bidf=bidf,
    )
    # schedule slots per window (identical across cores)
    slot_of_win = {}
    for t, w in enumerate(sched):
        slot_of_win.setdefault(w, []).append(t)

    in_maps = []
    for c, p in enumerate(plans):
        eids = p["eids"]
        blk0 = p["blk0"]
        idxJ = np.zeros((128, nt), np.int32)
        ilf = np.full((1, ns), 255.0, F16)   # 255 -> zero one-hot column
        spc = np.zeros((S_IN, ns), F16)
        slotmap = np.full(ns, -1, np.int64)
        used = {w: 0 for w in range(NWIN)}
        for (s, cnt, w) in p["tiles"]:
            t = slot_of_win[w][used[w]]
            used[w] += 1
            sl = eids[s:s + cnt]
            idxJ[:cnt, t] = jj_all[sl]
            ilf[0, t * TILE:t * TILE + cnt] = (ii_all[sl] % 128).astype(F16)
            spc[:, t * TILE:t * TILE + cnt] = spT_full[:, sl]
            slotmap[t * TILE:t * TILE + cnt] = sl
        ciwin = np.zeros((128, NWIN), np.int32)
        for w in range(NWIN):
            g = (blk0 + w) * 128 + np.arange(128)
            ciwin[:, w] = np.minimum(g, N - 1)
        m = dict(shared)
        m["idxJ"] = idxJ
        m["ilf"] = ilf
        m["spc"] = spc
        m["ciwin"] = ciwin
        in_maps.append(m)
        p["slotmap"] = slotmap
    return in_maps


def _build_program(sched):
    import concourse.bass as bass
    import concourse.tile as tile
    from concourse import bacc, mybir
    from contextlib import ExitStack

    f16 = mybir.dt.float16
    f32 = mybir.dt.float32
    i32 = mybir.dt.int32
    i16 = mybir.dt.int16
    AF = mybir.ActivationFunctionType
    OP = mybir.AluOpType

    KC_H = NODE_DIM // 128
    KDIM_H = NODE_DIM
    KC_GB = COND_DIM // 128 + 1
    NB1 = 512
    NCH1 = (N + NB1 - 1) // NB1
    NBLK = N // 128
    nt = len(sched)
    ns = nt * TILE

    nc = bacc.Bacc(
        "TRN2", target_bir_lowering=False, debug=False,
        num_devices=NCORES, num_swdge_queues=1,
    )

    nfT = nc.dram_tensor("nfT", [KDIM_H, N], f16, kind="ExternalInput")
    wnA = nc.dram_tensor("wnA", [KDIM_H, 128], f16, kind="ExternalInput")
    condA = nc.dram_tensor("condA", [KC_GB * 128, B], f16, kind="ExternalInput")
    wcA = nc.dram_tensor("wcA", [KC_GB * 128, 256], f16, kind="ExternalInput")
    ws = nc.dram_tensor("ws", [S_IN, S_OUT], f16, kind="ExternalInput")
    bs = nc.dram_tensor("bs", [S_OUT, 1], f32, kind="ExternalInput")
    bnc = nc.dram_tensor("bnc", [128, 1], f32, kind="ExternalInput")
    wlhi = nc.dram_tensor("wlhi", [128, 128], f16, kind="ExternalInput")
    wlhj = nc.dram_tensor("wlhj", [128, 128], f16, kind="ExternalInput")
    wls = nc.dram_tensor("wls", [S_OUT, 128], f16, kind="ExternalInput")
    bidf = nc.dram_tensor("bidf", [1, N], f16, kind="ExternalInput")
    idxJ = nc.dram_tensor("idxJ", [128, nt], i32, kind="ExternalInput")
    ilf = nc.dram_tensor("ilf", [1, ns], f16, kind="ExternalInput")
    ciwin = nc.dram_tensor("ciwin", [128, NWIN], i32, kind="ExternalInput")
    spc = nc.dram_tensor("spc", [S_IN, ns], f16, kind="ExternalInput")
    out = nc.dram_tensor("out", [128, nt, 128], f16, kind="ExternalOutput")

    tbl = nc.dram_tensor("tbl", [N, 384], f16)

    with tile.TileContext(nc) as tc:
        with ExitStack() as ctx:
            const = ctx.enter_context(tc.tile_pool(name="const", bufs=1))

            wn_sb = const.tile([128, KC_H * 128], f16)
            for k in range(KC_H):
                nc.sync.dma_start(
                    out=wn_sb[:, k * 128:(k + 1) * 128],
                    in_=wnA.ap()[k * 128:(k + 1) * 128, :],
                )
            ones_sb = const.tile([1, 128], f16)
            nc.vector.memset(ones_sb[:], 1.0)
            cond_sb = const.tile([128, KC_GB * 128], f16)
            wc_sb = const.tile([128, KC_GB * 256], f16)
            for k in range(KC_GB):
                nc.sync.dma_start(
                    out=cond_sb[:, k * 128:(k + 1) * 128],
                    in_=condA.ap()[k * 128:(k + 1) * 128, :],
                )
                nc.sync.dma_start(
                    out=wc_sb[:, k * 256:(k + 1) * 256],
                    in_=wcA.ap()[k * 128:(k + 1) * 128, :],
                )
            ws_sb = const.tile([S_IN, S_OUT], f16)
            nc.sync.dma_start(out=ws_sb[:], in_=ws.ap())
            bs_sb = const.tile([S_OUT, 1], f32)
            nc.sync.dma_start(out=bs_sb[:], in_=bs.ap())
            bn_sb = const.tile([128, 1], f32)
            nc.sync.dma_start(out=bn_sb[:], in_=bnc.ap())
            wlhi_sb = const.tile([128, 128], f16)
            nc.sync.dma_start(out=wlhi_sb[:], in_=wlhi.ap())
            wlhj_sb = const.tile([128, 128], f16)
            nc.sync.dma_start(out=wlhj_sb[:], in_=wlhj.ap())
            wls_sb = const.tile([S_OUT, 128], f16)
            nc.sync.dma_start(out=wls_sb[:], in_=wls.ap())
            bid_sb = const.tile([1, N], f16)
            nc.sync.dma_start(out=bid_sb[:], in_=bidf.ap())
            idxj_sb = const.tile([128, nt], i32)
            nc.sync.dma_start(out=idxj_sb[:], in_=idxJ.ap())
            ciwin_sb = const.tile([128, NWIN], i32)
            nc.sync.dma_start(out=ciwin_sb[:], in_=ciwin.ap())
            eps_sb = const.tile([128, 1], f32)
            nc.vector.memset(eps_sb[:], EPS)
            io16 = const.tile([128, 1], i16)
            nc.gpsimd.iota(io16[:], pattern=[[0, 1]], channel_multiplier=1)
            iof = const.tile([128, 1], f32)
            nc.vector.tensor_copy(iof[:], io16[:])
            gb_sb = const.tile([128, 256], f16)
            ci_loc = const.tile([128, NWIN, 128], f16)

            # ================= phase 1: node table =================
            with ExitStack() as p1:
                w1 = p1.enter_context(tc.tile_pool(name="w1", bufs=2))
                ps1 = p1.enter_context(
                    tc.tile_pool(name="ps1", bufs=1, space="PSUM")
                )

                gb_ps = ps1.tile([128, 256], f32, tag="gbps")
                for k in range(KC_GB):
                    nc.tensor.matmul(
                        out=gb_ps[:],
                        lhsT=cond_sb[:, k * 128:(k + 1) * 128],
                        rhs=wc_sb[:, k * 256:(k + 1) * 256],
                        start=(k == 0), stop=(k == KC_GB - 1),
                    )
                nc.scalar.copy(gb_sb[:], gb_ps[:])

                # per-node batch one-hot source: broadcast bid row
                bidr = const.tile([128, N], f16)
                nc.gpsimd.partition_broadcast(bidr[:], bid_sb[:])
                ohb_all = const.tile([128, NBLK, 128], f16)
                nc.vector.tensor_scalar(
                    out=ohb_all[:].rearrange("p a b -> p (a b)"),
                    in0=bidr[:], scalar1=iof[:], scalar2=None,
                    op0=OP.is_equal,
                )

                ci_dram = nc.dram_tensor("ci_tbl", [N, 128], f16)

                for nb in range(NCH1):
                    n0 = nb * NB1
                    nsz = min(NB1, N - n0)
                    nts = nsz // 128
                    ht_psA = ps1.tile([128, NB1], f32, tag="htpsA", bufs=2)
                    ht_psB = ps1.tile([128, NB1], f32, tag="htpsB", bufs=2)
                    for k4 in range(KC_H // 4):
                        nf_t = w1.tile([128, 4, NB1], f16, tag="nft", bufs=4)
                        nc.sync.dma_start(
                            out=nf_t[:, :, :nsz],
                            in_=nfT.ap()[
                                k4 * 512:(k4 + 1) * 512, n0:n0 + nsz
                            ].rearrange("(a p) n -> p a n", p=128),
                        )
                        for kk in range(4):
                            k = k4 * 4 + kk
                            ps = ht_psA if k % 2 == 0 else ht_psB
                            nc.tensor.matmul(
                                out=ps[:, :nsz],
                                lhsT=wn_sb[:, k * 128:(k + 1) * 128],
                                rhs=nf_t[:, kk, :nsz],
                                start=(k < 2), stop=(k >= KC_H - 2),
                            )
                    ht_b = w1.tile([128, NB1], f32, tag="htb", bufs=2)
                    nc.vector.tensor_copy(ht_b[:, :nsz], ht_psB[:, :nsz])
                    ht_f = w1.tile([128, NB1], f32, tag="htf", bufs=2)
                    nc.vector.tensor_tensor(
                        out=ht_f[:, :nsz], in0=ht_psA[:, :nsz],
                        in1=ht_b[:, :nsz], op=OP.add,
                    )
                    ht_sb = w1.tile([128, NB1], f16, tag="htsb", bufs=3)
                    nc.scalar.activation(
                        ht_sb[:, :nsz], ht_f[:, :nsz], AF.Relu, bias=bn_sb[:]
                    )
                    for st in range(nts):
                        blk = (n0 + st * 128) // 128
                        lhs = ht_sb[:, st * 128:(st + 1) * 128]
                        tb_sb = w1.tile([128, 384], f16, tag="tbsb", bufs=3)
                        nb_ps = ps1.tile([128, 4, 128], f32, tag="nbps", bufs=2)
                        nc.tensor.matmul(
                            out=nb_ps[:, 0, :], lhsT=lhs, rhs=wlhj_sb[:],
                            start=True, stop=True,
                        )
                        nc.tensor.matmul(
                            out=nb_ps[:, 1:3, :].rearrange("p a b -> p (a b)"),
                            lhsT=ohb_all[:, blk, :],
                            rhs=gb_sb[:], start=True, stop=True,
                        )
                        nc.scalar.copy(
                            tb_sb[:],
                            nb_ps[:, 0:3, :].rearrange("p a b -> p (a b)"),
                        )
                        nc.sync.dma_start(
                            out=tbl.ap()[blk * 128:(blk + 1) * 128, :],
                            in_=tb_sb[:],
                        )
                        nc.tensor.matmul(
                            out=nb_ps[:, 3, :], lhsT=lhs, rhs=wlhi_sb[:],
                            start=True, stop=True,
                        )
                        ci_sb = w1.tile([128, 128], f16, tag="cisb", bufs=3)
                        nc.scalar.copy(ci_sb[:], nb_ps[:, 3, :])
                        nc.sync.dma_start(
                            out=ci_dram.ap()[blk * 128:(blk + 1) * 128, :],
                            in_=ci_sb[:],
                        )

                tc.strict_bb_all_engine_barrier()

                # own Ci windows -> SBUF via 8 single-column indirect DMAs
                for wdx in range(NWIN):
                    nc.gpsimd.indirect_dma_start(
                        out=ci_loc[:, wdx, :], out_offset=None,
                        in_=ci_dram.ap(),
                        in_offset=bass.IndirectOffsetOnAxis(
                            ap=ciwin_sb[:, wdx:wdx + 1], axis=0
                        ),
                    )

                tc.strict_bb_all_engine_barrier()

            # ================= phase 2: edges =================
            with ExitStack() as p2:
                w2 = p2.enter_context(tc.tile_pool(name="w2", bufs=2))
                sm = p2.enter_context(tc.tile_pool(name="sm", bufs=4))
                ps_x = p2.enter_context(
                    tc.tile_pool(name="psx", bufs=2, space="PSUM")
                )
                ps_s = p2.enter_context(
                    tc.tile_pool(name="pss", bufs=2, space="PSUM")
                )
                ps_r = p2.enter_context(
                    tc.tile_pool(name="psr", bufs=2, space="PSUM")
                )

                NGRP = nt // GRP
                for g in range(NGRP):
                    t0 = g * GRP
                    if t0 % CH == 0:
                        ils = w2.tile([1, CH * TILE], f16, tag="ils")
                        nc.sync.dma_start(
                            out=ils[:],
                            in_=ilf.ap()[:, t0 * TILE:(t0 + CH) * TILE],
                        )
                        ohi = w2.tile([128, CH, TILE], f16, tag="ohi")
                        for q in range(CH * TILE // 512):
                            ilr_ps = ps_r.tile([128, 512], f32, tag="ilr")
                            nc.tensor.matmul(
                                out=ilr_ps[:], lhsT=ones_sb[:],
                                rhs=ils[:, q * 512:(q + 1) * 512],
                                start=True, stop=True,
                            )
                            nc.vector.tensor_scalar(
                                out=ohi[:, q * 4:(q + 1) * 4, :].rearrange(
                                    "p a b -> p (a b)"
                                ),
                                in0=ilr_ps[:], scalar1=iof[:], scalar2=None,
                                op0=OP.is_equal,
                            )
                        spt = w2.tile([S_IN, CH * TILE], f16, tag="spt")
                        nc.sync.dma_start(
                            out=spt[:],
                            in_=spc.ap()[:, t0 * TILE:(t0 + CH) * TILE],
                        )
                        cje = w2.tile([128, CH, 384], f16, tag="cje")
                        ob = w2.tile([128, CH, 128], f16, tag="ob")
                    co = t0 % CH

                    for j in range(GRP):
                        nc.gpsimd.indirect_dma_start(
                            out=cje[:, co + j, :], out_offset=None,
                            in_=tbl.ap(),
                            in_offset=bass.IndirectOffsetOnAxis(
                                ap=idxj_sb[:, t0 + j:t0 + j + 1], axis=0
                            ),
                        )

                    s_ps = ps_s.tile([S_OUT, GRP * TILE], f32, tag="sps")
                    nc.tensor.matmul(
                        out=s_ps[:], lhsT=ws_sb[:],
                        rhs=spt[:, co * TILE:(co + GRP) * TILE],
                        start=True, stop=True,
                    )
                    sT = sm.tile([S_OUT, GRP * TILE], f16, tag="sT")
                    nc.scalar.activation(sT[:], s_ps[:], AF.Relu, bias=bs_sb[:])

                    xs_ps = ps_x.tile([128, GRP, 128], f32, tag="xs")
                    for j in range(GRP):
                        nc.tensor.matmul(
                            out=xs_ps[:, j, :],
                            lhsT=ohi[:, co + j, :],
                            rhs=ci_loc[:, sched[t0 + j], :],
                            start=True, stop=False,
                        )
                        nc.tensor.matmul(
                            out=xs_ps[:, j, :],
                            lhsT=sT[:, j * TILE:(j + 1) * TILE],
                            rhs=wls_sb[:], start=False, stop=True,
                        )

                    xsb = sm.tile([128, GRP, 128], f16, tag="xsb")
                    nc.vector.tensor_tensor(
                        out=xsb[:], in0=cje[:, co:co + GRP, 0:128],
                        in1=xs_ps[:], op=OP.add,
                    )
                    sums = sm.tile([128, GRP], f32, tag="sums")
                    nc.vector.tensor_reduce(
                        out=sums[:], in_=xsb[:], axis=mybir.AxisListType.X,
                        op=OP.add,
                    )
                    sq = sm.tile([128, GRP, 128], f16, tag="sq")
                    nc.scalar.square(sq[:], xsb[:])
                    ssq = sm.tile([128, GRP], f32, tag="ssq")
                    nc.vector.tensor_reduce(
                        out=ssq[:], in_=sq[:], axis=mybir.AxisListType.X,
                        op=OP.add,
                    )
                    negmu = sm.tile([128, GRP], f32, tag="negmu")
                    nc.vector.tensor_scalar(
                        out=negmu[:], in0=sums[:], scalar1=-1.0 / 128,
                        scalar2=None, op0=OP.mult,
                    )
                    musq = sm.tile([128, GRP], f32, tag="musq")
                    nc.vector.tensor_tensor(
                        out=musq[:], in0=negmu[:], in1=negmu[:], op=OP.mult,
                    )
                    var = sm.tile([128, GRP], f32, tag="var")
                    nc.vector.tensor_scalar(
                        out=var[:], in0=ssq[:], scalar1=1.0 / 128,
                        scalar2=None, op0=OP.mult,
                    )
                    nc.vector.tensor_tensor(
                        out=var[:], in0=var[:], in1=musq[:], op=OP.subtract,
                    )
                    std = sm.tile([128, GRP], f32, tag="std")
                    nc.scalar.activation(
                        std[:], var[:], AF.Sqrt, bias=eps_sb[:]
                    )
                    rstd = sm.tile([128, GRP], f32, tag="rstd")
                    nc.vector.reciprocal(rstd[:], std[:])
                    nmr = sm.tile([128, GRP], f32, tag="nmr")
                    nc.vector.tensor_tensor(
                        out=nmr[:], in0=negmu[:], in1=rstd[:], op=OP.mult,
                    )
                    xn = sm.tile([128, GRP, 128], f16, tag="xn")
                    for j in range(GRP):
                        if j % 2 == 0:
                            nc.scalar.activation(
                                xn[:, j, :], xsb[:, j, :], AF.Identity,
                                bias=nmr[:, j:j + 1], scale=rstd[:, j:j + 1],
                            )
                        else:
                            nc.vector.tensor_scalar(
                                out=xn[:, j, :], in0=xsb[:, j, :],
                                scalar1=negmu[:, j:j + 1],
                                scalar2=rstd[:, j:j + 1],
                                op0=OP.add, op1=OP.mult,
                            )
                    xf = sm.tile([128, GRP, 128], f16, tag="xf")
                    nc.vector.tensor_tensor(
                        out=xf[:], in0=xn[:], in1=cje[:, co:co + GRP, 128:256],
                        op=OP.mult,
                    )
                    xb = sm.tile([128, GRP, 128], f16, tag="xb")
                    nc.vector.tensor_tensor(
                        out=xb[:], in0=xf[:], in1=cje[:, co:co + GRP, 256:384],
                        op=OP.add,
                    )
                    nc.scalar.activation(ob[:, co:co + GRP, :], xb[:], AF.Relu)
                    if (t0 + GRP) % CH == 0:
                        tc0 = t0 + GRP - CH
                        nc.sync.dma_start(
                            out=out.ap()[:, tc0:tc0 + CH, :],
                            in_=ob[:],
                        )

    nc.compile()
    return nc


def _run(inputs, trace=False):
    _axon_shim()
    from concourse.bass_utils import run_bass_kernel_spmd

    ii = np.asarray(inputs["node_i_ids"])
    key = hash(ii.tobytes())
    if _cache.get("key") != key:
        plans, sched = _plan(inputs)
        _cache.update(
            key=key, plans=plans, sched=sched, nc=_build_program(sched)
        )
    plans, sched, nc = _cache["plans"], _cache["sched"], _cache["nc"]
    in_maps = _prep_inputs(inputs, plans, sched)

    res = run_bass_kernel_spmd(
        nc, in_maps, core_ids=list(range(NCORES)), trace=trace
    )
    full = np.zeros((E, 128), np.float32)
    for c, p in enumerate(plans):
        sm_ = p["slotmap"]
        valid = sm_ >= 0
        o = res.results[c]["out"].transpose(1, 0, 2).reshape(-1, 128)
        full[sm_[valid]] = o[valid].astype(np.float32)
    return full, res


def kernel(**inputs):
    full, _ = _run(inputs, trace=False)
    return full.astype(np.float32)


# revision 36
# speedup vs baseline: 1.1301x; 1.0459x over previous
"""CondGraphConv Trainium2 kernel: 8-core SPMD, edge-sharded (i-sorted).

Algebraic restructuring:
    x_e  = Ci[i_e] + Cj[j_e] + relu(sp_e @ Ws + bs) @ Wl_s
    out_e = relu(LN(x_e) * gamma[bid[j_e]] + beta[bid[j_e]])
  where Ci = h @ Wl[:128], Cj = h @ Wl[128:256], h = relu(nf @ Wn + bn).

Phase 1 builds a per-node DRAM table with rows [Cj(128)|gamma(128)|beta(128)]
(768B; gamma/beta expanded per node by an on-chip one-hot matmul over
batch_ids) and keeps each core's own 8 Ci windows resident in SBUF.
Phase 2 processes 128-edge tiles cut at i-window boundaries: the i-side
contribution is a one-hot matmul against the resident Ci window (one-hot
built on-chip: partition_broadcast + is_equal vs an iota column); the j-side
[Cj|gamma|beta] arrives via one indirect DMA per tile (single-column offsets
are the only HW-supported form).  LN mean/var via segmented reduces and an
Act-engine square; normalize+FiLM+relu split across Act/DVE.  Output f16;
host inverse-permutes and upcasts.
"""

import sys
import types

for _p in ("/opt/trn_rl_repo",):
    if _p not in sys.path:
        sys.path.append(_p)

import numpy as np

N, E, B = 6400, 313600, 128
NODE_DIM, COND_DIM, EDGE_DIM = 2048, 1024, 128
S_IN, S_OUT = 8, 30
EPS = 1e-5

NCORES = 8
ECORE = E // NCORES           # 39200 edges per core
NLOC = N // NCORES            # 800 own i-nodes per core
TILE = 128
CH = 16                       # tiles per chunk
GRP = 4                       # tiles per group
F16 = np.float16

_cache = {}


def _axon_shim():
    try:
        import antenv.axon_hooks  # noqa: F401
        return
    except ImportError:
        pass
    try:
        import antenv
        from trn_agent_boot.trn_boot import _ntff_profile_via_ctypes
    except ImportError:
        return
    mod = types.ModuleType("antenv.axon_hooks")
    holder = [None]
    mod.set_axon_ntff_profile_hook = lambda h: holder.__setitem__(0, h)
    mod.get_axon_ntff_profile_hook = lambda: holder[0]
    sys.modules["antenv.axon_hooks"] = mod
    antenv.axon_hooks = mod
    try:
        mod.set_axon_ntff_profile_hook(
            _ntff_profile_via_ctypes("/opt/axon/libaxon_pjrt.so")
        )
    except Exception:
        pass


NWIN = 8
NSH = 896                     # phase-1 nodes per core (7 blocks)
NFULL = NSH * NCORES          # 7168 padded table rows


def _plan(inputs):
    """Shard edges by i-range (equal slices of the i-sorted list), cut each
    core's edges into <=128 tiles that never span a 128-node i-window
    (window = global block relative to the core's first block), and build a
    core-uniform tile->window schedule (padded with per-window dummies)."""
    ii = np.asarray(inputs["node_i_ids"]).astype(np.int64)
    order = np.argsort(ii, kind="stable")
    plans = []
    counts = np.zeros((NCORES, NWIN), np.int64)
    for c in range(NCORES):
        eids = order[c * ECORE:(c + 1) * ECORE]
        blk = ii[eids] // 128          # global window id, non-decreasing
        blk0 = int(blk[0])
        tiles = []                      # (start, cnt, win_rel)
        s = 0
        ne = eids.shape[0]
        while s < ne:
            b = blk[s]
            e = min(s + TILE, ne)
            e = s + int(np.searchsorted(blk[s:e], b + 1))
            w = int(b - blk0)
            assert 0 <= w < NWIN
            tiles.append((s, e - s, w))
            counts[c, w] += 1
            s = e
        plans.append({"eids": eids, "blk0": blk0, "tiles": tiles})
    maxcnt = counts.max(axis=0)
    sched = []
    for w in range(NWIN):
        sched.extend([w] * int(maxcnt[w]))
    while len(sched) % CH:
        sched.append(NWIN - 1)
    return plans, sched


def _prep_inputs(inputs, plans, sched):
    nt = len(sched)
    ns = nt * TILE

    nf = np.asarray(inputs["node_feats"], np.float32)
    nfT_full = nf.T.astype(F16)          # [2048, N]
    wnA = np.asarray(inputs["Wn"], np.float32).astype(F16)
    bnc = np.asarray(inputs["bn"], np.float32).reshape(128, 1)

    KC_GB = COND_DIM // 128 + 1
    KDIM_GB = KC_GB * 128
    cond = np.asarray(inputs["cond_feats"], np.float32)
    condA = np.zeros((KDIM_GB, B), F16)
    condA[:COND_DIM] = cond.T.astype(F16)
    condA[COND_DIM] = 1.0
    wcA = np.zeros((KDIM_GB, 256), F16)
    wcA[:COND_DIM] = np.asarray(inputs["Wc"], np.float32).astype(F16)
    bc_plus = np.asarray(inputs["bc"], np.float32).copy()
    bc_plus[:EDGE_DIM] += 1.0
    wcA[COND_DIM] = bc_plus.astype(F16)

    ws = np.asarray(inputs["Ws"], np.float32).astype(F16)
    bs = np.asarray(inputs["bs"], np.float32).reshape(S_OUT, 1)
    wl = np.asarray(inputs["Wl"], np.float32)
    wlhi = wl[:128].astype(F16)
    wlhj = wl[128:256].astype(F16)
    wls = wl[256:].astype(F16)

    bid = np.asarray(inputs["batch_ids"]).astype(np.int64)
    bidf = bid.astype(F16).reshape(1, N)
    jj_all = np.asarray(inputs["node_j_ids"]).astype(np.int64)
    ii_all = np.asarray(inputs["node_i_ids"]).astype(np.int64)
    spT_full = np.asarray(inputs["spatial_feats"], np.float32).T.astype(F16)

    shared = dict(
        wnA=wnA, condA=condA, wcA=wcA, ws=ws, bs=bs, bnc=bnc,
        wlhi=wlhi, wlhj=wlhj, wls=wls,
    )
    # schedule slots per window (identical across cores)
    slot_of_win = {}
    for t, w in enumerate(sched):
        slot_of_win.setdefault(w, []).append(t)

    in_maps = []
    for c, p in enumerate(plans):
        eids = p["eids"]
        blk0 = p["blk0"]
        idxJ = np.zeros((128, nt), np.int32)
        ilf = np.full((1, ns), 255.0, F16)   # 255 -> zero one-hot column
        spc = np.zeros((S_IN, ns), F16)
        slotmap = np.full(ns, -1, np.int64)
        used = {w: 0 for w in range(NWIN)}
        for (s, cnt, w) in p["tiles"]:
            t = slot_of_win[w][used[w]]
            used[w] += 1
            sl = eids[s:s + cnt]
            idxJ[:cnt, t] = jj_all[sl]
            ilf[0, t * TILE:t * TILE + cnt] = (ii_all[sl] % 128).astype(F16)
            spc[:, t * TILE:t * TILE + cnt] = spT_full[:, sl]
            slotmap[t * TILE:t * TILE + cnt] = sl
        ciwin = np.zeros((128, NWIN), np.int32)
        for w in range(NWIN):
            g = (blk0 + w) * 128 + np.arange(128)
            ciwin[:, w] = np.minimum(g, N - 1)
        m = dict(shared)
        n0 = c * NSH
        n1 = min(n0 + NSH, N)
        nfT_sh = np.zeros((NODE_DIM, NSH), F16)
        nfT_sh[:, :n1 - n0] = nfT_full[:, n0:n1]
        bid_sh = np.zeros((1, NSH), F16)
        bid_sh[0, :n1 - n0] = bidf[0, n0:n1]
        m["nfT"] = nfT_sh
        m["bidf"] = bid_sh
        m["idxJ"] = idxJ
        m["ilf"] = ilf
        m["spc"] = spc
        m["ciwin"] = ciwin
        in_maps.append(m)
        p["slotmap"] = slotmap
    return in_maps


def _build_program(sched):
    import concourse.bass as bass
    import concourse.tile as tile
    from concourse import bacc, mybir
    from contextlib import ExitStack

    f16 = mybir.dt.float16
    f32 = mybir.dt.float32
    i32 = mybir.dt.int32
    i16 = mybir.dt.int16
    AF = mybir.ActivationFunctionType
    OP = mybir.AluOpType

    KC_H = NODE_DIM // 128
    KDIM_H = NODE_DIM
    KC_GB = COND_DIM // 128 + 1
    NB1 = 512
    NCH1 = (NSH + NB1 - 1) // NB1
    NBLK = NSH // 128
    nt = len(sched)
    ns = nt * TILE

    nc = bacc.Bacc(
        "TRN2", target_bir_lowering=False, debug=False,
        num_devices=NCORES, num_swdge_queues=1,
    )

    nfT = nc.dram_tensor("nfT", [KDIM_H, NSH], f16, kind="ExternalInput")
    wnA = nc.dram_tensor("wnA", [KDIM_H, 128], f16, kind="ExternalInput")
    condA = nc.dram_tensor("condA", [KC_GB * 128, B], f16, kind="ExternalInput")
    wcA = nc.dram_tensor("wcA", [KC_GB * 128, 256], f16, kind="ExternalInput")
    ws = nc.dram_tensor("ws", [S_IN, S_OUT], f16, kind="ExternalInput")
    bs = nc.dram_tensor("bs", [S_OUT, 1], f32, kind="ExternalInput")
    bnc = nc.dram_tensor("bnc", [128, 1], f32, kind="ExternalInput")
    wlhi = nc.dram_tensor("wlhi", [128, 128], f16, kind="ExternalInput")
    wlhj = nc.dram_tensor("wlhj", [128, 128], f16, kind="ExternalInput")
    wls = nc.dram_tensor("wls", [S_OUT, 128], f16, kind="ExternalInput")
    bidf = nc.dram_tensor("bidf", [1, NSH], f16, kind="ExternalInput")
    idxJ = nc.dram_tensor("idxJ", [128, nt], i32, kind="ExternalInput")
    ilf = nc.dram_tensor("ilf", [1, ns], f16, kind="ExternalInput")
    ciwin = nc.dram_tensor("ciwin", [128, NWIN], i32, kind="ExternalInput")
    spc = nc.dram_tensor("spc", [S_IN, ns], f16, kind="ExternalInput")
    out = nc.dram_tensor("out", [128, nt, 128], f16, kind="ExternalOutput")

    tbl_sh = nc.dram_tensor("tbl_sh", [NSH, 384], f16)
    tbl = nc.dram_tensor("tbl", [NFULL, 384], f16, addr_space="Shared")
    ci_sh = nc.dram_tensor("ci_sh", [NSH, 128], f16)

    with tile.TileContext(nc) as tc:
        with ExitStack() as ctx:
            const = ctx.enter_context(tc.tile_pool(name="const", bufs=1))

            wn_sb = const.tile([128, KC_H * 128], f16)
            for k in range(KC_H):
                nc.sync.dma_start(
                    out=wn_sb[:, k * 128:(k + 1) * 128],
                    in_=wnA.ap()[k * 128:(k + 1) * 128, :],
                )
            ones_sb = const.tile([1, 128], f16)
            nc.vector.memset(ones_sb[:], 1.0)
            cond_sb = const.tile([128, KC_GB * 128], f16)
            wc_sb = const.tile([128, KC_GB * 256], f16)
            for k in range(KC_GB):
                nc.sync.dma_start(
                    out=cond_sb[:, k * 128:(k + 1) * 128],
                    in_=condA.ap()[k * 128:(k + 1) * 128, :],
                )
                nc.sync.dma_start(
                    out=wc_sb[:, k * 256:(k + 1) * 256],
                    in_=wcA.ap()[k * 128:(k + 1) * 128, :],
                )
            ws_sb = const.tile([S_IN, S_OUT], f16)
            nc.sync.dma_start(out=ws_sb[:], in_=ws.ap())
            bs_sb = const.tile([S_OUT, 1], f32)
            nc.sync.dma_start(out=bs_sb[:], in_=bs.ap())
            bn_sb = const.tile([128, 1], f32)
            nc.sync.dma_start(out=bn_sb[:], in_=bnc.ap())
            wlhi_sb = const.tile([128, 128], f16)
            nc.sync.dma_start(out=wlhi_sb[:], in_=wlhi.ap())
            wlhj_sb = const.tile([128, 128], f16)
            nc.sync.dma_start(out=wlhj_sb[:], in_=wlhj.ap())
            wls_sb = const.tile([S_OUT, 128], f16)
            nc.sync.dma_start(out=wls_sb[:], in_=wls.ap())
            bid_sb = const.tile([1, NSH], f16)
            nc.sync.dma_start(out=bid_sb[:], in_=bidf.ap())
            idxj_sb = const.tile([128, nt], i32)
            nc.sync.dma_start(out=idxj_sb[:], in_=idxJ.ap())
            ciwin_sb = const.tile([128, NWIN], i32)
            nc.sync.dma_start(out=ciwin_sb[:], in_=ciwin.ap())
            eps_sb = const.tile([128, 1], f32)
            nc.vector.memset(eps_sb[:], EPS)
            io16 = const.tile([128, 1], i16)
            nc.gpsimd.iota(io16[:], pattern=[[0, 1]], channel_multiplier=1)
            iof = const.tile([128, 1], f32)
            nc.vector.tensor_copy(iof[:], io16[:])
            gb_sb = const.tile([128, 256], f16)
            ci_loc = const.tile([128, NWIN, 128], f16)

            # ================= phase 1: node table =================
            with ExitStack() as p1:
                w1 = p1.enter_context(tc.tile_pool(name="w1", bufs=2))
                ps1 = p1.enter_context(
                    tc.tile_pool(name="ps1", bufs=1, space="PSUM")
                )

                gb_ps = ps1.tile([128, 256], f32, tag="gbps")
                for k in range(KC_GB):
                    nc.tensor.matmul(
                        out=gb_ps[:],
                        lhsT=cond_sb[:, k * 128:(k + 1) * 128],
                        rhs=wc_sb[:, k * 256:(k + 1) * 256],
                        start=(k == 0), stop=(k == KC_GB - 1),
                    )
                nc.scalar.copy(gb_sb[:], gb_ps[:])

                # per-node batch one-hot source: broadcast bid row
                bidr = const.tile([128, NSH], f16)
                nc.gpsimd.partition_broadcast(bidr[:], bid_sb[:])
                ohb_all = const.tile([128, NSH // 128, 128], f16)
                nc.vector.tensor_scalar(
                    out=ohb_all[:].rearrange("p a b -> p (a b)"),
                    in0=bidr[:], scalar1=iof[:], scalar2=None,
                    op0=OP.is_equal,
                )

                ci_dram = nc.dram_tensor("ci_tbl", [NFULL, 128], f16, addr_space="Shared")

                for nb in range(NCH1):
                    n0 = nb * NB1
                    nsz = min(NB1, NSH - n0)
                    nts = nsz // 128
                    ht_psA = ps1.tile([128, NB1], f32, tag="htpsA", bufs=2)
                    ht_psB = ps1.tile([128, NB1], f32, tag="htpsB", bufs=2)
                    for k4 in range(KC_H // 4):
                        nf_t = w1.tile([128, 4, NB1], f16, tag="nft", bufs=4)
                        nc.sync.dma_start(
                            out=nf_t[:, :, :nsz],
                            in_=nfT.ap()[
                                k4 * 512:(k4 + 1) * 512, n0:n0 + nsz
                            ].rearrange("(a p) n -> p a n", p=128),
                        )
                        for kk in range(4):
                            k = k4 * 4 + kk
                            ps = ht_psA if k % 2 == 0 else ht_psB
                            nc.tensor.matmul(
                                out=ps[:, :nsz],
                                lhsT=wn_sb[:, k * 128:(k + 1) * 128],
                                rhs=nf_t[:, kk, :nsz],
                                start=(k < 2), stop=(k >= KC_H - 2),
                            )
                    ht_b = w1.tile([128, NB1], f32, tag="htb", bufs=2)
                    nc.vector.tensor_copy(ht_b[:, :nsz], ht_psB[:, :nsz])
                    ht_f = w1.tile([128, NB1], f32, tag="htf", bufs=2)
                    nc.vector.tensor_tensor(
                        out=ht_f[:, :nsz], in0=ht_psA[:, :nsz],
                        in1=ht_b[:, :nsz], op=OP.add,
                    )
                    ht_sb = w1.tile([128, NB1], f16, tag="htsb", bufs=3)
                    nc.scalar.activation(
                        ht_sb[:, :nsz], ht_f[:, :nsz], AF.Relu, bias=bn_sb[:]
                    )
                    for st in range(nts):
                        blk = (n0 + st * 128) // 128
                        lhs = ht_sb[:, st * 128:(st + 1) * 128]
                        tb_sb = w1.tile([128, 384], f16, tag="tbsb", bufs=3)
                        nb_ps = ps1.tile([128, 4, 128], f32, tag="nbps", bufs=2)
                        nc.tensor.matmul(
                            out=nb_ps[:, 0, :], lhsT=lhs, rhs=wlhj_sb[:],
                            start=True, stop=True,
                        )
                        nc.tensor.matmul(
                            out=nb_ps[:, 1:3, :].rearrange("p a b -> p (a b)"),
                            lhsT=ohb_all[:, blk, :],
                            rhs=gb_sb[:], start=True, stop=True,
                        )
                        nc.scalar.copy(
                            tb_sb[:],
                            nb_ps[:, 0:3, :].rearrange("p a b -> p (a b)"),
                        )
                        nc.sync.dma_start(
                            out=tbl_sh.ap()[blk * 128:(blk + 1) * 128, :],
                            in_=tb_sb[:],
                        )
                        nc.tensor.matmul(
                            out=nb_ps[:, 3, :], lhsT=lhs, rhs=wlhi_sb[:],
                            start=True, stop=True,
                        )
                        ci_sb = w1.tile([128, 128], f16, tag="cisb", bufs=3)
                        nc.scalar.copy(ci_sb[:], nb_ps[:, 3, :])
                        nc.sync.dma_start(
                            out=ci_sh.ap()[blk * 128:(blk + 1) * 128, :],
                            in_=ci_sb[:],
                        )

                nc.gpsimd.collective_compute(
                    kind="AllGather", op=OP.bypass,
                    replica_groups=[list(range(NCORES))],
                    ins=[tbl_sh.ap()], outs=[tbl.ap()],
                )
                nc.gpsimd.collective_compute(
                    kind="AllGather", op=OP.bypass,
                    replica_groups=[list(range(NCORES))],
                    ins=[ci_sh.ap()], outs=[ci_dram.ap()],
                )

                tc.strict_bb_all_engine_barrier()

                # own Ci windows -> SBUF via 8 single-column indirect DMAs
                for wdx in range(NWIN):
                    nc.gpsimd.indirect_dma_start(
                        out=ci_loc[:, wdx, :], out_offset=None,
                        in_=ci_dram.ap(),
                        in_offset=bass.IndirectOffsetOnAxis(
                            ap=ciwin_sb[:, wdx:wdx + 1], axis=0
                        ),
                    )

                tc.strict_bb_all_engine_barrier()

            # ================= phase 2: edges =================
            with ExitStack() as p2:
                w2 = p2.enter_context(tc.tile_pool(name="w2", bufs=2))
                sm = p2.enter_context(tc.tile_pool(name="sm", bufs=4))
                ps_x = p2.enter_context(
                    tc.tile_pool(name="psx", bufs=2, space="PSUM")
                )
                ps_s = p2.enter_context(
                    tc.tile_pool(name="pss", bufs=2, space="PSUM")
                )
                ps_r = p2.enter_context(
                    tc.tile_pool(name="psr", bufs=2, space="PSUM")
                )

                NGRP = nt // GRP
                for g in range(NGRP):
                    t0 = g * GRP
                    if t0 % CH == 0:
                        ils = w2.tile([1, CH * TILE], f16, tag="ils")
                        nc.sync.dma_start(
                            out=ils[:],
                            in_=ilf.ap()[:, t0 * TILE:(t0 + CH) * TILE],
                        )
                        ohi = w2.tile([128, CH, TILE], f16, tag="ohi")
                        for q in range(CH * TILE // 512):
                            ilr_ps = ps_r.tile([128, 512], f32, tag="ilr")
                            nc.tensor.matmul(
                                out=ilr_ps[:], lhsT=ones_sb[:],
                                rhs=ils[:, q * 512:(q + 1) * 512],
                                start=True, stop=True,
                            )
                            nc.vector.tensor_scalar(
                                out=ohi[:, q * 4:(q + 1) * 4, :].rearrange(
                                    "p a b -> p (a b)"
                                ),
                                in0=ilr_ps[:], scalar1=iof[:], scalar2=None,
                                op0=OP.is_equal,
                            )
                        spt = w2.tile([S_IN, CH * TILE], f16, tag="spt")
                        nc.sync.dma_start(
                            out=spt[:],
                            in_=spc.ap()[:, t0 * TILE:(t0 + CH) * TILE],
                        )
                        cje = w2.tile([128, CH, 384], f16, tag="cje", bufs=3)
                        ob = w2.tile([128, CH, 128], f16, tag="ob")
                    co = t0 % CH

                    for j in range(GRP):
                        nc.gpsimd.indirect_dma_start(
                            out=cje[:, co + j, :], out_offset=None,
                            in_=tbl.ap(),
                            in_offset=bass.IndirectOffsetOnAxis(
                                ap=idxj_sb[:, t0 + j:t0 + j + 1], axis=0
                            ),
                        )

                    s_ps = ps_s.tile([S_OUT, GRP * TILE], f32, tag="sps")
                    nc.tensor.matmul(
                        out=s_ps[:], lhsT=ws_sb[:],
                        rhs=spt[:, co * TILE:(co + GRP) * TILE],
                        start=True, stop=True,
                    )
                    sT = sm.tile([S_OUT, GRP * TILE], f16, tag="sT")
                    nc.scalar.activation(sT[:], s_ps[:], AF.Relu, bias=bs_sb[:])

                    xs_ps = ps_x.tile([128, GRP, 128], f32, tag="xs")
                    for j in range(GRP):
                        nc.tensor.matmul(
                            out=xs_ps[:, j, :],
                            lhsT=ohi[:, co + j, :],
                            rhs=ci_loc[:, sched[t0 + j], :],
                            start=True, stop=False,
                        )
                        nc.tensor.matmul(
                            out=xs_ps[:, j, :],
                            lhsT=sT[:, j * TILE:(j + 1) * TILE],
                            rhs=wls_sb[:], start=False, stop=True,
                        )

                    xsb = sm.tile([128, GRP, 128], f16, tag="xsb")
                    nc.vector.tensor_tensor(
                        out=xsb[:], in0=cje[:, co:co + GRP, 0:128],
                        in1=xs_ps[:], op=OP.add,
                    )
                    sums = sm.tile([128, GRP], f32, tag="sums")
                    nc.vector.tensor_reduce(
                        out=sums[:], in_=xsb[:], axis=mybir.AxisListType.X,
                        op=OP.add,
                    )
                    sq = sm.tile([128, GRP, 128], f16, tag="sq")
                    nc.scalar.square(sq[:], xsb[:])
                    ssq = sm.tile([128, GRP], f32, tag="ssq")
                    nc.vector.tensor_reduce(
                        out=ssq[:], in_=sq[:], axis=mybir.AxisListType.X,
                        op=OP.add,
                    )
                    negmu = sm.tile([128, GRP], f32, tag="negmu")
                    nc.vector.tensor_scalar(
                        out=negmu[:], in0=sums[:], scalar1=-1.0 / 128,
                        scalar2=None, op0=OP.mult,
                    )
                    musq = sm.tile([128, GRP], f32, tag="musq")
                    nc.vector.tensor_tensor(
                        out=musq[:], in0=negmu[:], in1=negmu[:], op=OP.mult,
                    )
                    var = sm.tile([128, GRP], f32, tag="var")
                    nc.vector.tensor_scalar(
                        out=var[:], in0=ssq[:], scalar1=1.0 / 128,
                        scalar2=None, op0=OP.mult,
                    )
                    nc.vector.tensor_tensor(
                        out=var[:], in0=var[:], in1=musq[:], op=OP.subtract,
                    )
                    std = sm.tile([128, GRP], f32, tag="std")
                    nc.scalar.activation(
                        std[:], var[:], AF.Sqrt, bias=eps_sb[:]
                    )
                    rstd = sm.tile([128, GRP], f32, tag="rstd")
                    nc.vector.reciprocal(rstd[:], std[:])
                    nmr = sm.tile([128, GRP], f32, tag="nmr")
                    nc.vector.tensor_tensor(
                        out=nmr[:], in0=negmu[:], in1=rstd[:], op=OP.mult,
                    )
                    xn = sm.tile([128, GRP, 128], f16, tag="xn")
                    for j in range(GRP):
                        if j % 2 == 0:
                            nc.scalar.activation(
                                xn[:, j, :], xsb[:, j, :], AF.Identity,
                                bias=nmr[:, j:j + 1], scale=rstd[:, j:j + 1],
                            )
                        else:
                            nc.vector.tensor_scalar(
                                out=xn[:, j, :], in0=xsb[:, j, :],
                                scalar1=negmu[:, j:j + 1],
                                scalar2=rstd[:, j:j + 1],
                                op0=OP.add, op1=OP.mult,
                            )
                    xf = sm.tile([128, GRP, 128], f16, tag="xf")
                    nc.vector.tensor_tensor(
                        out=xf[:], in0=xn[:], in1=cje[:, co:co + GRP, 128:256],
                        op=OP.mult,
                    )
                    xb = sm.tile([128, GRP, 128], f16, tag="xb")
                    nc.vector.tensor_tensor(
                        out=xb[:], in0=xf[:], in1=cje[:, co:co + GRP, 256:384],
                        op=OP.add,
                    )
                    nc.scalar.activation(ob[:, co:co + GRP, :], xb[:], AF.Relu)
                    if (t0 + GRP) % CH == 0:
                        tc0 = t0 + GRP - CH
                        nc.sync.dma_start(
                            out=out.ap()[:, tc0:tc0 + CH, :],
                            in_=ob[:],
                        )

    nc.compile()
    return nc


def _run(inputs, trace=False):
    _axon_shim()
    from concourse.bass_utils import run_bass_kernel_spmd

    ii = np.asarray(inputs["node_i_ids"])
    key = hash(ii.tobytes())
    if _cache.get("key") != key:
        plans, sched = _plan(inputs)
        _cache.update(
            key=key, plans=plans, sched=sched, nc=_build_program(sched)
        )
    plans, sched, nc = _cache["plans"], _cache["sched"], _cache["nc"]
    in_maps = _prep_inputs(inputs, plans, sched)

    res = run_bass_kernel_spmd(
        nc, in_maps, core_ids=list(range(NCORES)), trace=trace
    )
    full = np.zeros((E, 128), np.float32)
    for c, p in enumerate(plans):
        sm_ = p["slotmap"]
        valid = sm_ >= 0
        o = res.results[c]["out"].transpose(1, 0, 2).reshape(-1, 128)
        full[sm_[valid]] = o[valid].astype(np.float32)
    return full, res


def kernel(**inputs):
    full, _ = _run(inputs, trace=False)
    return full.astype(np.float32)


# revision 37
# speedup vs baseline: 1.1545x; 1.0216x over previous
"""CondGraphConv Trainium2 kernel: 8-core SPMD, edge-sharded (i-sorted).

Algebraic restructuring:
    x_e  = Ci[i_e] + Cj[j_e] + relu(sp_e @ Ws + bs) @ Wl_s
    out_e = relu(LN(x_e) * gamma[bid[j_e]] + beta[bid[j_e]])
  where Ci = h @ Wl[:128], Cj = h @ Wl[128:256], h = relu(nf @ Wn + bn).

Phase 1 builds a per-node DRAM table with rows [Cj(128)|gamma(128)|beta(128)]
(768B; gamma/beta expanded per node by an on-chip one-hot matmul over
batch_ids) and keeps each core's own 8 Ci windows resident in SBUF.
Phase 2 processes 128-edge tiles cut at i-window boundaries: the i-side
contribution is a one-hot matmul against the resident Ci window (one-hot
built on-chip: partition_broadcast + is_equal vs an iota column); the j-side
[Cj|gamma|beta] arrives via one indirect DMA per tile (single-column offsets
are the only HW-supported form).  LN mean/var via segmented reduces and an
Act-engine square; normalize+FiLM+relu split across Act/DVE.  Output f16;
host inverse-permutes and upcasts.
"""

import sys
import types

for _p in ("/opt/trn_rl_repo",):
    if _p not in sys.path:
        sys.path.append(_p)

import numpy as np

N, E, B = 6400, 313600, 128
NODE_DIM, COND_DIM, EDGE_DIM = 2048, 1024, 128
S_IN, S_OUT = 8, 30
EPS = 1e-5

NCORES = 8
ECORE = E // NCORES           # 39200 edges per core
NLOC = N // NCORES            # 800 own i-nodes per core
TILE = 128
CH = 16                       # tiles per chunk
GRP = 4                       # tiles per group
F16 = np.float16

_cache = {}


def _axon_shim():
    try:
        import antenv.axon_hooks  # noqa: F401
        return
    except ImportError:
        pass
    try:
        import antenv
        from trn_agent_boot.trn_boot import _ntff_profile_via_ctypes
    except ImportError:
        return
    mod = types.ModuleType("antenv.axon_hooks")
    holder = [None]
    mod.set_axon_ntff_profile_hook = lambda h: holder.__setitem__(0, h)
    mod.get_axon_ntff_profile_hook = lambda: holder[0]
    sys.modules["antenv.axon_hooks"] = mod
    antenv.axon_hooks = mod
    try:
        mod.set_axon_ntff_profile_hook(
            _ntff_profile_via_ctypes("/opt/axon/libaxon_pjrt.so")
        )
    except Exception:
        pass


NWIN = 8
NSH = 896                     # phase-1 nodes per core (7 blocks)
NFULL = NSH * NCORES          # 7168 padded table rows


def _plan(inputs):
    """Shard edges by i-range (equal slices of the i-sorted list), cut each
    core's edges into <=128 tiles that never span a 128-node i-window
    (window = global block relative to the core's first block), and build a
    core-uniform tile->window schedule (padded with per-window dummies)."""
    ii = np.asarray(inputs["node_i_ids"]).astype(np.int64)
    order = np.argsort(ii, kind="stable")
    plans = []
    counts = np.zeros((NCORES, NWIN), np.int64)
    for c in range(NCORES):
        eids = order[c * ECORE:(c + 1) * ECORE]
        blk = ii[eids] // 128          # global window id, non-decreasing
        blk0 = int(blk[0])
        tiles = []                      # (start, cnt, win_rel)
        s = 0
        ne = eids.shape[0]
        while s < ne:
            b = blk[s]
            e = min(s + TILE, ne)
            e = s + int(np.searchsorted(blk[s:e], b + 1))
            w = int(b - blk0)
            assert 0 <= w < NWIN
            tiles.append((s, e - s, w))
            counts[c, w] += 1
            s = e
        plans.append({"eids": eids, "blk0": blk0, "tiles": tiles})
    maxcnt = counts.max(axis=0)
    sched = []
    for w in range(NWIN):
        sched.extend([w] * int(maxcnt[w]))
    while len(sched) % CH:
        sched.append(NWIN - 1)
    return plans, sched


def _prep_inputs(inputs, plans, sched):
    nt = len(sched)
    ns = nt * TILE

    nf = np.asarray(inputs["node_feats"], np.float32)
    nfT_full = nf.T.astype(F16)          # [2048, N]
    wnA = np.asarray(inputs["Wn"], np.float32).astype(F16)
    bnc = np.asarray(inputs["bn"], np.float32).reshape(128, 1)

    KC_GB = COND_DIM // 128 + 1
    KDIM_GB = KC_GB * 128
    cond = np.asarray(inputs["cond_feats"], np.float32)
    condA = np.zeros((KDIM_GB, B), F16)
    condA[:COND_DIM] = cond.T.astype(F16)
    condA[COND_DIM] = 1.0
    wcA = np.zeros((KDIM_GB, 256), F16)
    wcA[:COND_DIM] = np.asarray(inputs["Wc"], np.float32).astype(F16)
    bc_plus = np.asarray(inputs["bc"], np.float32).copy()
    bc_plus[:EDGE_DIM] += 1.0
    wcA[COND_DIM] = bc_plus.astype(F16)

    ws = np.asarray(inputs["Ws"], np.float32).astype(F16)
    bs = np.asarray(inputs["bs"], np.float32).reshape(S_OUT, 1)
    wl = np.asarray(inputs["Wl"], np.float32)
    wlhi = wl[:128].astype(F16)
    wlhj = wl[128:256].astype(F16)
    wls = wl[256:].astype(F16)

    bid = np.asarray(inputs["batch_ids"]).astype(np.int64)
    bidf = bid.astype(F16).reshape(1, N)
    jj_all = np.asarray(inputs["node_j_ids"]).astype(np.int64)
    ii_all = np.asarray(inputs["node_i_ids"]).astype(np.int64)
    spT_full = np.asarray(inputs["spatial_feats"], np.float32).T.astype(F16)

    shared = dict(
        wnA=wnA, condA=condA, wcA=wcA, ws=ws, bs=bs, bnc=bnc,
        wlhi=wlhi, wlhj=wlhj, wls=wls,
    )
    # schedule slots per window (identical across cores)
    slot_of_win = {}
    for t, w in enumerate(sched):
        slot_of_win.setdefault(w, []).append(t)

    in_maps = []
    for c, p in enumerate(plans):
        eids = p["eids"]
        blk0 = p["blk0"]
        idxJ = np.zeros((128, nt), np.int32)
        ilf = np.full((1, ns), 255.0, F16)   # 255 -> zero one-hot column
        spc = np.zeros((S_IN, ns), F16)
        slotmap = np.full(ns, -1, np.int64)
        used = {w: 0 for w in range(NWIN)}
        for (s, cnt, w) in p["tiles"]:
            t = slot_of_win[w][used[w]]
            used[w] += 1
            sl = eids[s:s + cnt]
            idxJ[:cnt, t] = jj_all[sl]
            ilf[0, t * TILE:t * TILE + cnt] = (ii_all[sl] % 128).astype(F16)
            spc[:, t * TILE:t * TILE + cnt] = spT_full[:, sl]
            slotmap[t * TILE:t * TILE + cnt] = sl
        ciwin = np.zeros((128, NWIN), np.int32)
        for w in range(NWIN):
            g = (blk0 + w) * 128 + np.arange(128)
            ciwin[:, w] = np.minimum(g, N - 1)
        m = dict(shared)
        n0 = c * NSH
        n1 = min(n0 + NSH, N)
        nfT_sh = np.zeros((NODE_DIM, NSH), F16)
        nfT_sh[:, :n1 - n0] = nfT_full[:, n0:n1]
        bid_sh = np.zeros((1, NSH), F16)
        bid_sh[0, :n1 - n0] = bidf[0, n0:n1]
        m["nfT"] = nfT_sh
        m["bidf"] = bid_sh
        m["idxJ"] = idxJ
        m["ilf"] = ilf
        m["spc"] = spc
        m["ciwin"] = ciwin
        in_maps.append(m)
        p["slotmap"] = slotmap
    return in_maps


def _build_program(sched):
    import concourse.bass as bass
    import concourse.tile as tile
    from concourse import bacc, mybir
    from contextlib import ExitStack

    f16 = mybir.dt.float16
    f32 = mybir.dt.float32
    i32 = mybir.dt.int32
    i16 = mybir.dt.int16
    AF = mybir.ActivationFunctionType
    OP = mybir.AluOpType

    KC_H = NODE_DIM // 128
    KDIM_H = NODE_DIM
    KC_GB = COND_DIM // 128 + 1
    NB1 = 512
    NCH1 = (NSH + NB1 - 1) // NB1
    NBLK = NSH // 128
    nt = len(sched)
    ns = nt * TILE

    nc = bacc.Bacc(
        "TRN2", target_bir_lowering=False, debug=False,
        num_devices=NCORES, num_swdge_queues=1,
    )

    nfT = nc.dram_tensor("nfT", [KDIM_H, NSH], f16, kind="ExternalInput")
    wnA = nc.dram_tensor("wnA", [KDIM_H, 128], f16, kind="ExternalInput")
    condA = nc.dram_tensor("condA", [KC_GB * 128, B], f16, kind="ExternalInput")
    wcA = nc.dram_tensor("wcA", [KC_GB * 128, 256], f16, kind="ExternalInput")
    ws = nc.dram_tensor("ws", [S_IN, S_OUT], f16, kind="ExternalInput")
    bs = nc.dram_tensor("bs", [S_OUT, 1], f32, kind="ExternalInput")
    bnc = nc.dram_tensor("bnc", [128, 1], f32, kind="ExternalInput")
    wlhi = nc.dram_tensor("wlhi", [128, 128], f16, kind="ExternalInput")
    wlhj = nc.dram_tensor("wlhj", [128, 128], f16, kind="ExternalInput")
    wls = nc.dram_tensor("wls", [S_OUT, 128], f16, kind="ExternalInput")
    bidf = nc.dram_tensor("bidf", [1, NSH], f16, kind="ExternalInput")
    idxJ = nc.dram_tensor("idxJ", [128, nt], i32, kind="ExternalInput")
    ilf = nc.dram_tensor("ilf", [1, ns], f16, kind="ExternalInput")
    ciwin = nc.dram_tensor("ciwin", [128, NWIN], i32, kind="ExternalInput")
    spc = nc.dram_tensor("spc", [S_IN, ns], f16, kind="ExternalInput")
    out = nc.dram_tensor("out", [128, nt, 128], f16, kind="ExternalOutput")

    tbl_sh = nc.dram_tensor("tbl_sh", [NSH, 512], f16)
    tbl = nc.dram_tensor("tbl", [NFULL, 512], f16, addr_space="Shared")

    with tile.TileContext(nc) as tc:
        with ExitStack() as ctx:
            const = ctx.enter_context(tc.tile_pool(name="const", bufs=1))

            wn_sb = const.tile([128, KC_H * 128], f16)
            for k in range(KC_H):
                nc.sync.dma_start(
                    out=wn_sb[:, k * 128:(k + 1) * 128],
                    in_=wnA.ap()[k * 128:(k + 1) * 128, :],
                )
            ones_sb = const.tile([1, 128], f16)
            nc.vector.memset(ones_sb[:], 1.0)
            cond_sb = const.tile([128, KC_GB * 128], f16)
            wc_sb = const.tile([128, KC_GB * 256], f16)
            for k in range(KC_GB):
                nc.sync.dma_start(
                    out=cond_sb[:, k * 128:(k + 1) * 128],
                    in_=condA.ap()[k * 128:(k + 1) * 128, :],
                )
                nc.sync.dma_start(
                    out=wc_sb[:, k * 256:(k + 1) * 256],
                    in_=wcA.ap()[k * 128:(k + 1) * 128, :],
                )
            ws_sb = const.tile([S_IN, S_OUT], f16)
            nc.sync.dma_start(out=ws_sb[:], in_=ws.ap())
            bs_sb = const.tile([S_OUT, 1], f32)
            nc.sync.dma_start(out=bs_sb[:], in_=bs.ap())
            bn_sb = const.tile([128, 1], f32)
            nc.sync.dma_start(out=bn_sb[:], in_=bnc.ap())
            wlhi_sb = const.tile([128, 128], f16)
            nc.sync.dma_start(out=wlhi_sb[:], in_=wlhi.ap())
            wlhj_sb = const.tile([128, 128], f16)
            nc.sync.dma_start(out=wlhj_sb[:], in_=wlhj.ap())
            wls_sb = const.tile([S_OUT, 128], f16)
            nc.sync.dma_start(out=wls_sb[:], in_=wls.ap())
            bid_sb = const.tile([1, NSH], f16)
            nc.sync.dma_start(out=bid_sb[:], in_=bidf.ap())
            idxj_sb = const.tile([128, nt], i32)
            nc.sync.dma_start(out=idxj_sb[:], in_=idxJ.ap())
            ciwin_sb = const.tile([128, NWIN], i32)
            nc.sync.dma_start(out=ciwin_sb[:], in_=ciwin.ap())
            eps_sb = const.tile([128, 1], f32)
            nc.vector.memset(eps_sb[:], EPS)
            io16 = const.tile([128, 1], i16)
            nc.gpsimd.iota(io16[:], pattern=[[0, 1]], channel_multiplier=1)
            iof = const.tile([128, 1], f32)
            nc.vector.tensor_copy(iof[:], io16[:])
            gb_sb = const.tile([128, 256], f16)
            ci_loc = const.tile([128, NWIN, 128], f16)

            # ================= phase 1: node table =================
            with ExitStack() as p1:
                w1 = p1.enter_context(tc.tile_pool(name="w1", bufs=2))
                ps1 = p1.enter_context(
                    tc.tile_pool(name="ps1", bufs=1, space="PSUM")
                )

                gb_ps = ps1.tile([128, 256], f32, tag="gbps")
                for k in range(KC_GB):
                    nc.tensor.matmul(
                        out=gb_ps[:],
                        lhsT=cond_sb[:, k * 128:(k + 1) * 128],
                        rhs=wc_sb[:, k * 256:(k + 1) * 256],
                        start=(k == 0), stop=(k == KC_GB - 1),
                    )
                nc.scalar.copy(gb_sb[:], gb_ps[:])

                # per-node batch one-hot source: broadcast bid row
                bidr = const.tile([128, NSH], f16)
                nc.gpsimd.partition_broadcast(bidr[:], bid_sb[:])
                ohb_all = const.tile([128, NSH // 128, 128], f16)
                nc.vector.tensor_scalar(
                    out=ohb_all[:].rearrange("p a b -> p (a b)"),
                    in0=bidr[:], scalar1=iof[:], scalar2=None,
                    op0=OP.is_equal,
                )


                for nb in range(NCH1):
                    n0 = nb * NB1
                    nsz = min(NB1, NSH - n0)
                    nts = nsz // 128
                    ht_psA = ps1.tile([128, NB1], f32, tag="htpsA", bufs=2)
                    ht_psB = ps1.tile([128, NB1], f32, tag="htpsB", bufs=2)
                    for k4 in range(KC_H // 4):
                        nf_t = w1.tile([128, 4, NB1], f16, tag="nft", bufs=4)
                        nc.sync.dma_start(
                            out=nf_t[:, :, :nsz],
                            in_=nfT.ap()[
                                k4 * 512:(k4 + 1) * 512, n0:n0 + nsz
                            ].rearrange("(a p) n -> p a n", p=128),
                        )
                        for kk in range(4):
                            k = k4 * 4 + kk
                            ps = ht_psA if k % 2 == 0 else ht_psB
                            nc.tensor.matmul(
                                out=ps[:, :nsz],
                                lhsT=wn_sb[:, k * 128:(k + 1) * 128],
                                rhs=nf_t[:, kk, :nsz],
                                start=(k < 2), stop=(k >= KC_H - 2),
                            )
                    ht_b = w1.tile([128, NB1], f32, tag="htb", bufs=2)
                    nc.vector.tensor_copy(ht_b[:, :nsz], ht_psB[:, :nsz])
                    ht_f = w1.tile([128, NB1], f32, tag="htf", bufs=2)
                    nc.vector.tensor_tensor(
                        out=ht_f[:, :nsz], in0=ht_psA[:, :nsz],
                        in1=ht_b[:, :nsz], op=OP.add,
                    )
                    ht_sb = w1.tile([128, NB1], f16, tag="htsb", bufs=3)
                    nc.scalar.activation(
                        ht_sb[:, :nsz], ht_f[:, :nsz], AF.Relu, bias=bn_sb[:]
                    )
                    for st in range(nts):
                        blk = (n0 + st * 128) // 128
                        lhs = ht_sb[:, st * 128:(st + 1) * 128]
                        tb_sb = w1.tile([128, 512], f16, tag="tbsb", bufs=3)
                        nb_ps = ps1.tile([128, 4, 128], f32, tag="nbps", bufs=2)
                        nc.tensor.matmul(
                            out=nb_ps[:, 0, :], lhsT=lhs, rhs=wlhj_sb[:],
                            start=True, stop=True,
                        )
                        nc.tensor.matmul(
                            out=nb_ps[:, 1:3, :].rearrange("p a b -> p (a b)"),
                            lhsT=ohb_all[:, blk, :],
                            rhs=gb_sb[:], start=True, stop=True,
                        )
                        nc.tensor.matmul(
                            out=nb_ps[:, 3, :], lhsT=lhs, rhs=wlhi_sb[:],
                            start=True, stop=True,
                        )
                        nc.scalar.copy(
                            tb_sb[:],
                            nb_ps[:].rearrange("p a b -> p (a b)"),
                        )
                        nc.sync.dma_start(
                            out=tbl_sh.ap()[blk * 128:(blk + 1) * 128, :],
                            in_=tb_sb[:],
                        )

                nc.gpsimd.collective_compute(
                    kind="AllGather", op=OP.bypass,
                    replica_groups=[list(range(NCORES))],
                    ins=[tbl_sh.ap()], outs=[tbl.ap()],
                )

                # own Ci windows -> SBUF via 8 single-column indirect DMAs
                for wdx in range(NWIN):
                    nc.gpsimd.indirect_dma_start(
                        out=ci_loc[:, wdx, :], out_offset=None,
                        in_=tbl.ap(),
                        in_offset=bass.IndirectOffsetOnAxis(
                            ap=ciwin_sb[:, wdx:wdx + 1], axis=0
                        ),
                        element_offset=384,
                    )

            # ================= phase 2: edges =================
            with ExitStack() as p2:
                w2 = p2.enter_context(tc.tile_pool(name="w2", bufs=2))
                sm = p2.enter_context(tc.tile_pool(name="sm", bufs=4))
                ps_x = p2.enter_context(
                    tc.tile_pool(name="psx", bufs=2, space="PSUM")
                )
                ps_s = p2.enter_context(
                    tc.tile_pool(name="pss", bufs=2, space="PSUM")
                )
                ps_r = p2.enter_context(
                    tc.tile_pool(name="psr", bufs=2, space="PSUM")
                )

                NGRP = nt // GRP
                for g in range(NGRP):
                    t0 = g * GRP
                    if t0 % CH == 0:
                        ils = w2.tile([1, CH * TILE], f16, tag="ils")
                        nc.sync.dma_start(
                            out=ils[:],
                            in_=ilf.ap()[:, t0 * TILE:(t0 + CH) * TILE],
                        )
                        ohi = w2.tile([128, CH, TILE], f16, tag="ohi")
                        for q in range(CH * TILE // 512):
                            ilr_ps = ps_r.tile([128, 512], f32, tag="ilr")
                            nc.tensor.matmul(
                                out=ilr_ps[:], lhsT=ones_sb[:],
                                rhs=ils[:, q * 512:(q + 1) * 512],
                                start=True, stop=True,
                            )
                            nc.vector.tensor_scalar(
                                out=ohi[:, q * 4:(q + 1) * 4, :].rearrange(
                                    "p a b -> p (a b)"
                                ),
                                in0=ilr_ps[:], scalar1=iof[:], scalar2=None,
                                op0=OP.is_equal,
                            )
                        spt = w2.tile([S_IN, CH * TILE], f16, tag="spt")
                        nc.sync.dma_start(
                            out=spt[:],
                            in_=spc.ap()[:, t0 * TILE:(t0 + CH) * TILE],
                        )
                        cje = w2.tile([128, CH, 384], f16, tag="cje", bufs=3)
                        ob = w2.tile([128, CH, 128], f16, tag="ob")
                    co = t0 % CH

                    for j in range(GRP):
                        nc.gpsimd.indirect_dma_start(
                            out=cje[:, co + j, :], out_offset=None,
                            in_=tbl.ap(),
                            in_offset=bass.IndirectOffsetOnAxis(
                                ap=idxj_sb[:, t0 + j:t0 + j + 1], axis=0
                            ),
                        )

                    s_ps = ps_s.tile([S_OUT, GRP * TILE], f32, tag="sps")
                    nc.tensor.matmul(
                        out=s_ps[:], lhsT=ws_sb[:],
                        rhs=spt[:, co * TILE:(co + GRP) * TILE],
                        start=True, stop=True,
                    )
                    sT = sm.tile([S_OUT, GRP * TILE], f16, tag="sT")
                    nc.scalar.activation(sT[:], s_ps[:], AF.Relu, bias=bs_sb[:])

                    xs_ps = ps_x.tile([128, GRP, 128], f32, tag="xs")
                    for j in range(GRP):
                        nc.tensor.matmul(
                            out=xs_ps[:, j, :],
                            lhsT=ohi[:, co + j, :],
                            rhs=ci_loc[:, sched[t0 + j], :],
                            start=True, stop=False,
                        )
                        nc.tensor.matmul(
                            out=xs_ps[:, j, :],
                            lhsT=sT[:, j * TILE:(j + 1) * TILE],
                            rhs=wls_sb[:], start=False, stop=True,
                        )

                    xsb = sm.tile([128, GRP, 128], f16, tag="xsb")
                    nc.vector.tensor_tensor(
                        out=xsb[:], in0=cje[:, co:co + GRP, 0:128],
                        in1=xs_ps[:], op=OP.add,
                    )
                    sums = sm.tile([128, GRP], f32, tag="sums")
                    nc.vector.tensor_reduce(
                        out=sums[:], in_=xsb[:], axis=mybir.AxisListType.X,
                        op=OP.add,
                    )
                    sq = sm.tile([128, GRP, 128], f16, tag="sq")
                    nc.scalar.square(sq[:], xsb[:])
                    ssq = sm.tile([128, GRP], f32, tag="ssq")
                    nc.vector.tensor_reduce(
                        out=ssq[:], in_=sq[:], axis=mybir.AxisListType.X,
                        op=OP.add,
                    )
                    negmu = sm.tile([128, GRP], f32, tag="negmu")
                    nc.vector.tensor_scalar(
                        out=negmu[:], in0=sums[:], scalar1=-1.0 / 128,
                        scalar2=None, op0=OP.mult,
                    )
                    musq = sm.tile([128, GRP], f32, tag="musq")
                    nc.vector.tensor_tensor(
                        out=musq[:], in0=negmu[:], in1=negmu[:], op=OP.mult,
                    )
                    var = sm.tile([128, GRP], f32, tag="var")
                    nc.vector.tensor_scalar(
                        out=var[:], in0=ssq[:], scalar1=1.0 / 128,
                        scalar2=None, op0=OP.mult,
                    )
                    nc.vector.tensor_tensor(
                        out=var[:], in0=var[:], in1=musq[:], op=OP.subtract,
                    )
                    std = sm.tile([128, GRP], f32, tag="std")
                    nc.scalar.activation(
                        std[:], var[:], AF.Sqrt, bias=eps_sb[:]
                    )
                    rstd = sm.tile([128, GRP], f32, tag="rstd")
                    nc.vector.reciprocal(rstd[:], std[:])
                    nmr = sm.tile([128, GRP], f32, tag="nmr")
                    nc.vector.tensor_tensor(
                        out=nmr[:], in0=negmu[:], in1=rstd[:], op=OP.mult,
                    )
                    xn = sm.tile([128, GRP, 128], f16, tag="xn")
                    for j in range(GRP):
                        if j % 2 == 0:
                            nc.scalar.activation(
                                xn[:, j, :], xsb[:, j, :], AF.Identity,
                                bias=nmr[:, j:j + 1], scale=rstd[:, j:j + 1],
                            )
                        else:
                            nc.vector.tensor_scalar(
                                out=xn[:, j, :], in0=xsb[:, j, :],
                                scalar1=negmu[:, j:j + 1],
                                scalar2=rstd[:, j:j + 1],
                                op0=OP.add, op1=OP.mult,
                            )
                    xf = sm.tile([128, GRP, 128], f16, tag="xf")
                    nc.vector.tensor_tensor(
                        out=xf[:], in0=xn[:], in1=cje[:, co:co + GRP, 128:256],
                        op=OP.mult,
                    )
                    xb = sm.tile([128, GRP, 128], f16, tag="xb")
                    nc.vector.tensor_tensor(
                        out=xb[:], in0=xf[:], in1=cje[:, co:co + GRP, 256:384],
                        op=OP.add,
                    )
                    nc.scalar.activation(ob[:, co:co + GRP, :], xb[:], AF.Relu)
                    nc.sync.dma_start(
                        out=out.ap()[:, t0:t0 + GRP, :],
                        in_=ob[:, co:co + GRP, :],
                    )

    nc.compile()
    return nc


def _run(inputs, trace=False):
    _axon_shim()
    from concourse.bass_utils import run_bass_kernel_spmd

    ii = np.asarray(inputs["node_i_ids"])
    key = hash(ii.tobytes())
    if _cache.get("key") != key:
        plans, sched = _plan(inputs)
        _cache.update(
            key=key, plans=plans, sched=sched, nc=_build_program(sched)
        )
    plans, sched, nc = _cache["plans"], _cache["sched"], _cache["nc"]
    in_maps = _prep_inputs(inputs, plans, sched)

    res = run_bass_kernel_spmd(
        nc, in_maps, core_ids=list(range(NCORES)), trace=trace
    )
    full = np.zeros((E, 128), np.float32)
    for c, p in enumerate(plans):
        sm_ = p["slotmap"]
        valid = sm_ >= 0
        o = res.results[c]["out"].transpose(1, 0, 2).reshape(-1, 128)
        full[sm_[valid]] = o[valid].astype(np.float32)
    return full, res


def kernel(**inputs):
    full, _ = _run(inputs, trace=False)
    return full.astype(np.float32)
